# revision 1
# baseline (speedup 1.0000x reference)
"""DiffJPEG (quality=75) Bass kernel for Trainium2, 8-core data-parallel.

v2 pipeline per image — zero PE transposes, both transpose stages fused
into neighboring matmuls via the stationary operand:
  conv:  x f32 -> fp16 tiles (input precision: fp16, validated rel<0.01)
  A+T1:  t1 = (rowDCT+color @ X).T computed directly as
         X_block.T @ (w*255*BD).T per output block (fp16 matmuls),
         Y level shift (-362.039) folded into the t1 evac DC columns.
  B:     col-DCT (+col-pool for chroma), f32r 512-wide matmuls.
  Q:     q1 = P*recip (DVE), q2 = +MAGIC (ACT copy w/ float bias),
         q3 = (q2-MAGIC)*q -> fp16 (DVE stt); tables are [128,8] tiles
         broadcast along the free dim via stride-0 APs.
  C+T2:  t2 = cq.T @ IDCT-consts per block (fp16), +128 output level
         folded into the Y t2-evac bias (per-partition, ACT Identity).
  D:     col-IDCT + color + upsample folds, consts pre-scaled 1/255 so
         PSUM holds final pixels in [0,1]; fp16 matmuls, 512-wide.
  fin:   single (min 1, max 0) tensor_scalar per chunk, then DMA out.
"""
import os
import sys

sys.path.insert(0, "/opt/trn_rl_repo")

import numpy as np

_CONV_PAT = os.environ.get("KV_CONV", "adadadadadad")
_SCHED_D = os.environ.get("KV_SCHED_D", "serial")
_WARM = int(os.environ.get("KV_WARM", "110"))
_Q2ENG = os.environ.get("KV_Q2", "pool")
_FINPAT = os.environ.get("KV_FIN", "addd")
_PSA = int(os.environ.get("KV_PSA", "2"))
_PSB = int(os.environ.get("KV_PSB", "2"))
_PST = int(os.environ.get("KV_PST", "2"))
_PSD = int(os.environ.get("KV_PSD", "2"))
_OUTPAIR = os.environ.get("KV_OUTPAIR", "0") == "1"
_AT1MIX = os.environ.get("KV_AT1MIX", "0") == "1"
_PIPE = os.environ.get("KV_PIPE", "0") == "1"
_CFIRST = os.environ.get("KV_CFIRST", "0") == "1"
_T1YR = os.environ.get("KV_T1YR", "d")
_T1C = os.environ.get("KV_T1C", "a")
_B256 = int(os.environ.get("KV_B256", "2"))
_A256 = int(os.environ.get("KV_A256", "2"))

QUALITY = 75
FACTOR = (200.0 - 2.0 * QUALITY) / 100.0  # 0.5
MAGIC = np.float32(1.5 * 2.0 ** 23)
LS = np.float64(128.0 * 8.0 * 0.5 / np.sqrt(2.0))  # 362.0386719675...

Y_TABLE = np.array([
    [16, 11, 10, 16, 24, 40, 51, 61],
    [12, 12, 14, 19, 26, 58, 60, 55],
    [14, 13, 16, 24, 40, 57, 69, 56],
    [14, 17, 22, 29, 51, 87, 80, 62],
    [18, 22, 37, 56, 68, 109, 103, 77],
    [24, 35, 55, 64, 81, 104, 113, 92],
    [49, 64, 78, 87, 103, 121, 120, 101],
    [72, 92, 95, 98, 112, 100, 103, 99]], dtype=np.float64)

C_TABLE = np.array([
    [17, 18, 24, 47, 99, 99, 99, 99],
    [18, 21, 26, 66, 99, 99, 99, 99],
    [24, 26, 56, 99, 99, 99, 99, 99],
    [47, 66, 99, 99, 99, 99, 99, 99],
    [99, 99, 99, 99, 99, 99, 99, 99],
    [99, 99, 99, 99, 99, 99, 99, 99],
    [99, 99, 99, 99, 99, 99, 99, 99],
    [99, 99, 99, 99, 99, 99, 99, 99]], dtype=np.float64)

W_FWD = {
    "y": (0.299, 0.587, 0.114),
    "cb": (-0.168736, -0.331264, 0.5),
    "cr": (0.5, -0.418688, -0.081312),
}
W_BWD = {
    "r": {"cr": 1.402},
    "g": {"cb": -0.344136, "cr": -0.714136},
    "b": {"cb": 1.772},
}

N_CORES = 8
IMGS_PER_CORE = 2
H = W = 512


def _round_f32r(x):
    """Round f32 to the 12-explicit-mantissa-bit f32r grid (RNE)."""
    x = np.ascontiguousarray(x, dtype=np.float32)
    u = x.view(np.uint32).astype(np.uint64)
    drop = 11
    half = np.uint64(1 << (drop - 1))
    low = u & np.uint64((1 << drop) - 1)
    u_hi = u >> np.uint64(drop)
    up = (low > half) | ((low == half) & ((u_hi & np.uint64(1)) == 1))
    u2 = (u_hi + up.astype(np.uint64)) << np.uint64(drop)
    return (u2 & np.uint64(0xFFFFFFFF)).astype(np.uint32).view(np.float32)


def _dct_mat():
    xg = np.arange(8, dtype=np.float64)
    ug = np.arange(8, dtype=np.float64)
    Dm = 0.5 * np.cos((2.0 * xg[None, :] + 1.0) * ug[:, None] * np.pi / 16.0)
    Dm[0, :] *= 1.0 / np.sqrt(2.0)
    return Dm


def _constants():
    D8 = _dct_mat()
    BD128 = np.kron(np.eye(16), D8)  # [128,128]
    P = np.zeros((128, 256))
    idx = np.arange(128)
    P[idx, 2 * idx] = 0.5
    P[idx, 2 * idx + 1] = 0.5
    M = np.kron(np.eye(16), D8) @ P  # [128, 256] row-pool + DCT
    P0, P1 = M[:, :128], M[:, 128:]

    # f32r pack: B-stage stationaries
    b_y = _round_f32r(BD128.T)
    b_c_k0 = _round_f32r(P0.T)
    b_c_k1 = _round_f32r(P1.T)
    pack_r = np.concatenate([b_y, b_c_k0, b_c_k1], axis=1)  # [128, 384]

    # fp16 pack: CT2 moving consts
    bd = np.asarray(BD128, dtype=np.float16)
    cc0 = np.asarray(2.0 * P0, dtype=np.float16)
    cc1 = np.asarray(2.0 * P1, dtype=np.float16)
    qy16 = np.tile((Y_TABLE.T * FACTOR), (16, 1)).astype(np.float16)
    qc16 = np.tile((C_TABLE.T * FACTOR), (16, 1)).astype(np.float16)
    pack_h = np.concatenate([bd, cc0, cc1, qy16, qc16],
                            axis=1)  # [128, 400] fp16

    # f32 pack: quant tables [128,8] x4 + bias_y [128,1]
    qy = np.tile((Y_TABLE.T * FACTOR), (16, 1)).astype(np.float32)
    qc = np.tile((C_TABLE.T * FACTOR), (16, 1)).astype(np.float32)
    ry = (1.0 / qy).astype(np.float32)
    rc = (1.0 / qc).astype(np.float32)
    bias_y = np.zeros((128, 1), dtype=np.float32)
    bias_y[0::8, 0] = np.float32(LS)
    pack_f = np.concatenate([qy, ry, qc, rc, bias_y], axis=1)  # [128, 33]

    return (np.ascontiguousarray(pack_r, dtype=np.float32),
            np.ascontiguousarray(pack_h, dtype=np.float16),
            np.ascontiguousarray(pack_f, dtype=np.float32))


_PACK_R, _PACK_H, _PACK_F = _constants()
_PROGRAM = None
TRACE = False
LAST_RESULT = None


def _build_program():
    import concourse.bacc as bacc
    import concourse.mybir as mybir
    from concourse.tile import TileContext

    f32 = mybir.dt.float32
    f32r = mybir.dt.float32r
    f16 = mybir.dt.float16
    ACT_COPY = mybir.ActivationFunctionType.Copy
    ACT_IDENT = mybir.ActivationFunctionType.Identity
    ADD = mybir.AluOpType.add
    SUB = mybir.AluOpType.subtract
    MULT = mybir.AluOpType.mult
    MIN = mybir.AluOpType.min
    MAX = mybir.AluOpType.max

    nc = bacc.Bacc("TRN2", target_bir_lowering=False, debug=False,
                   num_devices=N_CORES)

    x_d = nc.dram_tensor("xc", [IMGS_PER_CORE, 3, H, W], f32,
                         kind="ExternalInput").ap()
    out_d = nc.dram_tensor("outc", [IMGS_PER_CORE, 3, H, W], f32,
                           kind="ExternalOutput").ap()
    packr_d = nc.dram_tensor("pack_r", list(_PACK_R.shape), f32,
                             kind="ExternalInput").ap()
    packh_d = nc.dram_tensor("pack_h", list(_PACK_H.shape), f16,
                             kind="ExternalInput").ap()
    packf_d = nc.dram_tensor("pack_f", list(_PACK_F.shape), f32,
                             kind="ExternalInput").ap()

    with TileContext(nc) as tc:
        with (
            tc.tile_pool(name="const", bufs=1) as cpool,
            tc.tile_pool(name="data", bufs=2) as dpool,
            tc.tile_pool(name="work", bufs=2) as wpool,
            tc.tile_pool(name="psA", bufs=_PSA, space="PSUM") as psA,
            tc.tile_pool(name="psB", bufs=_PSB, space="PSUM") as psB,
            tc.tile_pool(name="psT", bufs=_PST, space="PSUM") as psT,
            tc.tile_pool(name="psD", bufs=_PSD, space="PSUM") as psD,
        ):
            # ---- PE warmup: dummy matmuls while DMAs are in flight ----
            wu0 = cpool.tile([128, 16], f32, name="wu0")
            nc.gpsimd.memset(wu0[:], 1.0)
            wu = cpool.tile([128, 16], f32r, name="wu")
            nc.gpsimd.tensor_copy(wu[:], wu0[:])

            # ---- constant DMAs (ACT queue: keeps SP free for the input
            # stream and DVE free for the warmup chain) ----
            cr_t = cpool.tile([128, 384], f32r, name="cr_t")
            nc.scalar.dma_start(cr_t[:], packr_d.bitcast(f32r))
            ch_t = cpool.tile([128, 400], f16, name="ch_t")
            nc.scalar.dma_start(ch_t[:], packh_d)
            cf_t = cpool.tile([128, 33], f32, name="cf_t")
            nc.scalar.dma_start(cf_t[:], packf_d)

            cs = {
                "b_y": cr_t[:, 0:128],
                "b_c_k0": cr_t[:, 128:256],
                "b_c_k1": cr_t[:, 256:384],
                "bd": ch_t[:, 0:128],
                "cc0": ch_t[:, 128:256],
                "cc1": ch_t[:, 256:384],
                "qy16": ch_t[:, 384:392],
                "qc16": ch_t[:, 392:400],
                "qy": cf_t[:, 0:8],
                "ry": cf_t[:, 8:16],
                "qc": cf_t[:, 16:24],
                "rc": cf_t[:, 24:32],
                "bias_y": cf_t[:, 32:33],
            }

            wp = psA.tile([128, 256], f32, name="wp", tag="ps256",
                          bufs=_A256)
            for _ in range(_WARM):
                nc.tensor.matmul(wp[:16, 0:16], wu[:], wu[:], start=True,
                                 stop=True)

            # ---- on-chip generated fp16 consts ----
            def gen16(key, src_ap, factor, width=128):
                t = cpool.tile([128, width], f16, name=f"g_{key}")
                nc.vector.tensor_scalar_mul(t[:], src_ap, float(factor))
                cs[key] = t[:]

            for wname, wv in zip("rgb", W_FWD["y"]):
                gen16(f"ay_{wname}", cs["b_y"], wv * 255.0)
            for cn in ("cb", "cr"):
                for wname, wv in zip("rgb", W_FWD[cn]):
                    gen16(f"a_{cn}_{wname}_k0", cs["b_c_k0"][:, 0:64],
                          wv * 255.0, width=64)
                    gen16(f"a_{cn}_{wname}_k1", cs["b_c_k1"][:, 64:128],
                          wv * 255.0, width=64)
            gen16("b_c16_k0", cs["b_c_k0"], 1.0)
            gen16("b_c16_k1", cs["b_c_k1"], 1.0)
            gen16("dd_y", cs["bd"], 1.0 / 255.0)
            for och, terms in W_BWD.items():
                for cch, wv in terms.items():
                    for k in (0, 1):
                        gen16(f"d_{och}_{cch}_k{k}", cs[f"cc{k}"],
                              wv / 255.0)

            def bc8(key, reps):
                """broadcast a [128,8] table along new dim: [128,*reps,8]."""
                ap = cs[key]
                for _ in range(len(reps)):
                    ap = ap.unsqueeze(1)
                return ap.broadcast_to([128, *reps, 8])

            def mm(out_ps, lhsT_ap, rhs_ap, start, stop):
                nc.tensor.matmul(out_ps, lhsT_ap, rhs_ap,
                                 start=start, stop=stop)

            S = [{} for _ in range(IMGS_PER_CORE)]

            # ---------------- stages ----------------
            def st_load(img):
                xt = dpool.tile([128, 3, 4, W], f32, name=f"xt_{img}",
                                tag="xt")
                for k in range(4):
                    for ch in range(3):
                        nc.sync.dma_start(
                            xt[:, ch, k, :],
                            x_d[img, ch, 128 * k:128 * (k + 1), :])
                S[img]["xt"] = xt

            _M = {"a": "act", "d": "dve", "p": "pool"}
            CONV_ENG = tuple(_M[c] for c in _CONV_PAT)

            def st_conv(img, h):
                xt = S[img]["xt"]
                x16 = S[img].get("x16")
                if x16 is None:
                    x16 = dpool.tile([128, 3, 4, W], f16, name=f"x16_{img}",
                                     tag="x16")
                    S[img]["x16"] = x16
                for k in (2 * h, 2 * h + 1):
                    for ch in range(3):
                        eng = CONV_ENG[(k * 3 + ch) % len(CONV_ENG)]
                        src = xt[:, ch, k, :]
                        dst = x16[:, ch, k, :]
                        if eng == "act":
                            nc.scalar.activation(dst, src, ACT_COPY)
                        elif eng == "dve":
                            nc.vector.tensor_copy(dst, src)
                        else:
                            nc.gpsimd.tensor_copy(dst, src)

            def st_AT1y(img, j, h):
                """fused A+T1 for Y, col-chunk j, row-half h."""
                x16 = S[img]["x16"]
                t1 = S[img].setdefault("t1y", {}).get(j)
                if t1 is None:
                    t1 = wpool.tile([128, W], f32r, name=f"t1y_{img}_{j}",
                                    tag="t1y", bufs=8)
                    S[img]["t1y"][j] = t1
                pa = psA.tile([128, 256], f32, name=f"AT1y_{img}_{j}_{h}",
                              tag="ps256", bufs=_A256)
                for ii in range(2):
                    i = 2 * h + ii
                    for ci, wname in enumerate("rgb"):
                        mm(pa[:, 128 * ii:128 * (ii + 1)],
                           x16[:, ci, i, 128 * j:128 * (j + 1)],
                           cs[f"ay_{wname}"], ci == 0, ci == 2)
                pav = pa[:].rearrange("p (a b) -> p a b", b=8)
                t1v = t1[:, 256 * h:256 * (h + 1)]\
                    .rearrange("p (a b) -> p a b", b=8)
                nc.vector.tensor_scalar_add(t1v[:, :, 0], pav[:, :, 0],
                                            -float(LS))
                nc.vector.tensor_copy(t1v[:, :, 1:8], pav[:, :, 1:8])

            def st_AT1c(img, cn, jp, h):
                """fused A+T1 chroma: j-pair jp, row-half h (i == h)."""
                x16 = S[img]["x16"]
                t1 = S[img].setdefault("t1c", {}).get((cn, jp))
                if t1 is None:
                    t1 = wpool.tile([128, 2, 256], f16,
                                    name=f"t1c_{img}_{cn}_{jp}", tag="t1c",
                                    bufs=8)
                    S[img]["t1c"][(cn, jp)] = t1
                pa = psA.tile([128, 256], f32,
                              name=f"AT1c_{img}_{cn}_{jp}_{h}", tag="ps256",
                              bufs=_A256)
                pav = pa[:].rearrange("p (a b) -> p a b", b=128)
                i = h
                for jj in range(2):
                    j = 2 * jp + jj
                    for k in range(2):
                        for ci, wname in enumerate("rgb"):
                            mm(pav[:, jj, 64 * k:64 * (k + 1)],
                               x16[:, ci, 2 * i + k,
                                   128 * j:128 * (j + 1)],
                               cs[f"a_{cn}_{wname}_k{k}"],
                               ci == 0, ci == 2)
                nc.scalar.activation(t1[:, :, 128 * h:128 * (h + 1)],
                                     pav[:], ACT_COPY)

            def st_B1y(img, i, h):
                t1 = S[img]["t1y"][i]
                cq = S[img].setdefault("cqy", {}).get(i)
                if cq is None:
                    cq = wpool.tile([128, W], f16, name=f"cq_{img}_y_{i}",
                                    tag="cqy", bufs=8)
                    S[img]["cqy"][i] = cq
                pb = psB.tile([128, 256], f32, name=f"B_{img}_y_{i}_{h}",
                              tag="ps256b", bufs=_B256)
                mm(pb[:], cs["b_y"], t1[:, 256 * h:256 * (h + 1)],
                   True, True)
                pbv = pb[:].rearrange("p (a b) -> p a b", b=8)
                w1 = wpool.tile([128, 256], f32, name=f"q1_{img}_y_{i}_{h}",
                                tag="q1", bufs=4)
                w1v = w1[:].rearrange("p (a b) -> p a b", b=8)
                nc.vector.tensor_tensor(w1v, pbv, bc8("ry", (32,)), MULT)
                w2 = wpool.tile([128, 256], f16, name=f"q2_{img}_y_{i}_{h}",
                                tag="q2", bufs=4)
                nc.gpsimd.tensor_scalar(w2[:], w1[:], float(MAGIC),
                                        float(MAGIC), ADD, SUB)
                cqv = cq[:, 256 * h:256 * (h + 1)]\
                    .rearrange("p (a b) -> p a b", b=8)
                w2v = w2[:].rearrange("p (a b) -> p a b", b=8)
                nc.vector.tensor_tensor(cqv, w2v, bc8("qy16", (32,)), MULT)

            def st_B1c(img, cn, h):
                t1c = S[img]["t1c"]
                ci = ("cb", "cr").index(cn)
                cq = S[img].setdefault("cqc", {}).get(ci)
                if cq is None:
                    cq = wpool.tile([128, W], f16, name=f"cq_{img}_{cn}",
                                    tag="cqc", bufs=4)
                    S[img]["cqc"][ci] = cq
                pb0 = psB.tile([128, 256], f32, name=f"B_{img}_{cn}_{h}",
                               tag="ps256b", bufs=_B256)
                pb = pb0[:].rearrange("p (c a) -> p c a", c=2)
                for b in range(2):
                    for k in range(2):
                        mm(pb[:, b, :], cs[f"b_c16_k{k}"],
                           t1c[(cn, b)][:, k, 128 * h:128 * (h + 1)],
                           k == 0, k == 1)
                pbv = pb0[:].rearrange("p (a b) -> p a b", b=8)
                w1 = wpool.tile([128, 256], f32, name=f"q1_{img}_{cn}_{h}",
                                tag="q1c", bufs=4)
                w1v = w1[:].rearrange("p (a b) -> p a b", b=8)
                nc.vector.tensor_tensor(w1v, pbv, bc8("rc", (32,)), MULT)
                w2 = wpool.tile([128, 256], f16, name=f"q2_{img}_{cn}_{h}",
                                tag="q2c", bufs=4)
                nc.gpsimd.tensor_scalar(w2[:], w1[:], float(MAGIC),
                                        float(MAGIC), ADD, SUB)
                # cq layout [128, (b, u)]: write u-half h per b chunk
                cqf = cq[:].rearrange("p (c u) -> p c u", c=2)
                w2f = w2[:].rearrange("p (c u) -> p c u", c=2)
                for b in range(2):
                    cqv = cqf[:, b, 128 * h:128 * (h + 1)]\
                        .rearrange("p (a b) -> p a b", b=8)
                    w2v = w2f[:, b, :].rearrange("p (a b) -> p a b", b=8)
                    nc.vector.tensor_tensor(cqv, w2v, bc8("qc16", (16,)),
                                            MULT)

            def st_CT2y(img, j):
                cqy = S[img]["cqy"]
                pt = psT.tile([128, W], f32, name=f"CT2y_{img}_{j}",
                              tag="psT")
                for i in range(4):
                    mm(pt[:, 128 * i:128 * (i + 1)],
                       cqy[i][:, 128 * j:128 * (j + 1)], cs["bd"],
                       True, True)
                t2 = wpool.tile([128, W], f16, name=f"t2y_{img}_{j}",
                                tag="t2y", bufs=8)
                nc.scalar.activation(t2[:], pt[:], ACT_IDENT,
                                     bias=cs["bias_y"], scale=1.0)
                S[img].setdefault("t2y", {})[j] = t2

            def st_CT2c(img, cn, j):
                cq = S[img]["cqc"][("cb", "cr").index(cn)]
                cqv = cq[:].rearrange("p (c u) -> p c u", c=2)
                pt = psT.tile([128, W], f32, name=f"CT2c_{img}_{cn}_{j}",
                              tag="psT")
                for i in range(4):
                    mm(pt[:, 128 * i:128 * (i + 1)],
                       cqv[:, i // 2, 128 * j:128 * (j + 1)],
                       cs[f"cc{i % 2}"], True, True)
                t2 = wpool.tile([128, W], f16, name=f"t2c_{img}_{cn}_{j}",
                                tag="t2c", bufs=8)
                nc.scalar.activation(t2[:], pt[:], ACT_COPY)
                S[img].setdefault("t2c", {})[(cn, j)] = t2

            def st_D(img, och, chunks):
                t2y = S[img]["t2y"]
                t2c = S[img]["t2c"]
                if True:
                    oi = "rgb".index(och)
                    ot = S[img].get(f"ot_{och}")
                    if ot is None:
                        ot = dpool.tile([128, 4, W], f32,
                                        name=f"ot_{img}_{och}", tag="ot",
                                        bufs=3)
                        S[img][f"ot_{och}"] = ot
                    for i in chunks:
                        pd = psD.tile([128, W], f32,
                                      name=f"D_{img}_{och}_{i}", tag="psD")
                        terms = list(W_BWD[och].items())
                        mm(pd[:], cs["dd_y"], t2y[i][:], True, False)
                        for ti, (cch, _) in enumerate(terms):
                            mm(pd[:], cs[f"d_{och}_{cch}_k{i % 2}"],
                               t2c[(cch, i // 2)][:],
                               False, ti == len(terms) - 1)
                        fe_n = img * 12 + oi * 4 + i
                        if _FINPAT[fe_n % len(_FINPAT)] == "a":
                            fe = wpool.tile([128, W], f32,
                                            name=f"fin_{img}_{och}_{i}",
                                            tag="fin", bufs=2)
                            nc.scalar.activation(
                                fe[:], pd[:],
                                mybir.ActivationFunctionType.Relu)
                            nc.gpsimd.tensor_scalar_min(ot[:, i, :],
                                                        fe[:], 1.0)
                        else:
                            nc.vector.tensor_scalar(ot[:, i, :], pd[:],
                                                    1.0, 0.0, MIN, MAX)
                        if not _OUTPAIR:
                            nc.sync.dma_start(
                                out_d[img, oi, 128 * i:128 * (i + 1), :],
                                ot[:, i, :])
                        elif i % 2 == 1:
                            hh = i // 2
                            nc.sync.dma_start(
                                out_d[img, oi, 256 * hh:256 * (hh + 1), :]
                                .rearrange("(k p) w -> p k w", p=128),
                                ot[:, 2 * hh:2 * hh + 2, :])

            # ---------------- emission schedule ----------------
            st_load(0)
            st_load(1)

            def half_front(img, h):
                st_conv(img, h)
                for j in range(4):
                    st_AT1y(img, j, h)
                for cn in ("cb", "cr"):
                    for jp in range(2):
                        st_AT1c(img, cn, jp, h)
                for i in range(4):
                    st_B1y(img, i, h)
                st_B1c(img, "cb", h)
                st_B1c(img, "cr", h)

            def half_back(img, h):
                st_CT2y(img, 2 * h)
                st_CT2y(img, 2 * h + 1)
                st_CT2c(img, "cr", h)
                st_CT2c(img, "cb", h)
                st_D(img, "r", (2 * h, 2 * h + 1))
                st_D(img, "g", (2 * h, 2 * h + 1))
                st_D(img, "b", (2 * h, 2 * h + 1))

            _HS = os.environ.get("KV_HS", "f00.b00.f01.f10.b01.f11.b10.b11")
            for tok in _HS.split("."):
                fn = half_front if tok[0] == "f" else half_back
                fn(int(tok[1]), int(tok[2]))

    nc.compile()
    return nc


def kernel(x: np.ndarray) -> np.ndarray:
    global _PROGRAM, LAST_RESULT
    from concourse.bass_utils import run_bass_kernel_spmd

    x = np.ascontiguousarray(np.asarray(x, dtype=np.float32))
    assert x.shape == (N_CORES * IMGS_PER_CORE, 3, H, W)

    if _PROGRAM is None:
        _PROGRAM = _build_program()
    nc = _PROGRAM

    in_maps = []
    for c in range(N_CORES):
        m = {"xc": x[IMGS_PER_CORE * c:IMGS_PER_CORE * (c + 1)],
             "pack_r": _PACK_R, "pack_h": _PACK_H, "pack_f": _PACK_F}
        in_maps.append(m)

    res = run_bass_kernel_spmd(nc, in_maps, list(range(N_CORES)), trace=TRACE)
    LAST_RESULT = res
    out = np.concatenate([res.results[c]["outc"] for c in range(N_CORES)],
                         axis=0)
    return out



# revision 23
# speedup vs baseline: 1.1343x; 1.1343x over previous
"""DiffJPEG (quality=75) Bass kernel for Trainium2, 8-core data-parallel.

v3 pipeline per image (f16 I/O, fused stages, cost-model-shaped):
  host:  input cast f32->f16, output cast f16->f32 (halves DMA traffic).
  load:  6 DMAs per image [128,2,512] f16 straight into A-stage layout.
  A+T1:  per (row-chunk r, col-pair jp) one PSUM group [128,512] =
         2x[Y(128)|cb(64)|cr(64)]; 6 fp16 MMs (3 colors x 2 j),
         rhs = fused per-color consts [128,256]. Evac: one Y copy
         [128,2,128] -> t1Y, one chroma copy [128,2,2,64] -> t1c.
  B:     col-DCT f32r(Y)/f16(chroma) into [128,512] PSUM groups
         (i-pairs for Y, both-cn for chroma).
  Q:     q1 = P*recip (DVE tt, split DC columns via scalar_tensor_tensor
         with per-partition -1024 level-shift offset), q2 = f32-magic
         round (DVE ts, 4x f16), q3 = *q (DVE tt, 2x f16).
  C+T2:  t2 = cq.T @ IDCT-consts per block (fp16), +LS bias on Y evac.
  D:     col-IDCT + color + upsample folds, consts pre-scaled 1/255;
         fp16 matmuls, 512-wide, grouped-LDW emission.
  fin:   clip via single (min 1, max 0) tensor_scalar per chunk spread
         across DVE/Pool, then f16 DMA out.
"""
import os
import sys

sys.path.insert(0, "/opt/trn_rl_repo")

import numpy as np

_WARM = int(os.environ.get("KV_WARM", "60"))
_FINPAT = os.environ.get("KV_FIN", "d")
_AEVY = os.environ.get("KV_AEVY", "a")     # A-evac Y copy engine
_AEVC = os.environ.get("KV_AEVC", "a")     # A-evac chroma copy engine
_CEVY = os.environ.get("KV_CEVY", "a")     # C-evac Y engine
_CEVC = os.environ.get("KV_CEVC", "a")     # C-evac chroma engine
_PSA = int(os.environ.get("KV_PSA", "2"))
_PSB = int(os.environ.get("KV_PSB", "2"))
_PST = int(os.environ.get("KV_PST", "2"))
_PSD = int(os.environ.get("KV_PSD", "2"))
_HS = os.environ.get("KV_HS", "f00.f01.f10.b00.b01.f11.b10.b11")

QUALITY = 75
FACTOR = (200.0 - 2.0 * QUALITY) / 100.0  # 0.5
MAGIC = np.float32(1.5 * 2.0 ** 23)
LS2D = 1024.0  # 2D-DCT of the -128 level shift, lands on (DC,DC) coefs
LSC = np.float64(128.0 * 8.0 * 0.5 / np.sqrt(2.0))  # +128 recon bias (C)

Y_TABLE = np.array([
    [16, 11, 10, 16, 24, 40, 51, 61],
    [12, 12, 14, 19, 26, 58, 60, 55],
    [14, 13, 16, 24, 40, 57, 69, 56],
    [14, 17, 22, 29, 51, 87, 80, 62],
    [18, 22, 37, 56, 68, 109, 103, 77],
    [24, 35, 55, 64, 81, 104, 113, 92],
    [49, 64, 78, 87, 103, 121, 120, 101],
    [72, 92, 95, 98, 112, 100, 103, 99]], dtype=np.float64)

C_TABLE = np.array([
    [17, 18, 24, 47, 99, 99, 99, 99],
    [18, 21, 26, 66, 99, 99, 99, 99],
    [24, 26, 56, 99, 99, 99, 99, 99],
    [47, 66, 99, 99, 99, 99, 99, 99],
    [99, 99, 99, 99, 99, 99, 99, 99],
    [99, 99, 99, 99, 99, 99, 99, 99],
    [99, 99, 99, 99, 99, 99, 99, 99],
    [99, 99, 99, 99, 99, 99, 99, 99]], dtype=np.float64)

W_FWD = {
    "y": (0.299, 0.587, 0.114),
    "cb": (-0.168736, -0.331264, 0.5),
    "cr": (0.5, -0.418688, -0.081312),
}
W_BWD = {
    "r": {"cr": 1.402},
    "g": {"cb": -0.344136, "cr": -0.714136},
    "b": {"cb": 1.772},
}

N_CORES = 8
IMGS_PER_CORE = 2
H = W = 512


def _round_f32r(x):
    """Round f32 to the 12-explicit-mantissa-bit f32r grid (RNE)."""
    x = np.ascontiguousarray(x, dtype=np.float32)
    u = x.view(np.uint32).astype(np.uint64)
    drop = 11
    half = np.uint64(1 << (drop - 1))
    low = u & np.uint64((1 << drop) - 1)
    u_hi = u >> np.uint64(drop)
    up = (low > half) | ((low == half) & ((u_hi & np.uint64(1)) == 1))
    u2 = (u_hi + up.astype(np.uint64)) << np.uint64(drop)
    return (u2 & np.uint64(0xFFFFFFFF)).astype(np.uint32).view(np.float32)


def _dct_mat():
    xg = np.arange(8, dtype=np.float64)
    ug = np.arange(8, dtype=np.float64)
    Dm = 0.5 * np.cos((2.0 * xg[None, :] + 1.0) * ug[:, None] * np.pi / 16.0)
    Dm[0, :] *= 1.0 / np.sqrt(2.0)
    return Dm


def _constants():
    D8 = _dct_mat()
    BD128 = np.kron(np.eye(16), D8)  # [128,128]
    P = np.zeros((128, 256))
    idx = np.arange(128)
    P[idx, 2 * idx] = 0.5
    P[idx, 2 * idx + 1] = 0.5
    M = np.kron(np.eye(16), D8) @ P  # [128, 256] row-pool + DCT
    P0, P1 = M[:, :128], M[:, 128:]

    # f32r pack: B-stage stationary for Y + A-const sources
    b_y = _round_f32r(BD128.T)
    b_c_k0 = _round_f32r(P0.T)
    b_c_k1 = _round_f32r(P1.T)
    pack_r = np.concatenate([b_y, b_c_k0, b_c_k1], axis=1)  # [128, 384]

    # fp16 pack: CT2 moving consts + B chroma stationaries + q tables
    bd = np.asarray(BD128, dtype=np.float16)
    cc0 = np.asarray(2.0 * P0, dtype=np.float16)
    cc1 = np.asarray(2.0 * P1, dtype=np.float16)
    bc0 = np.asarray(P0.T, dtype=np.float16)
    bc1 = np.asarray(P1.T, dtype=np.float16)
    qy16 = np.tile((Y_TABLE.T * FACTOR), (16, 1)).astype(np.float16)
    qc16 = np.tile((C_TABLE.T * FACTOR), (16, 1)).astype(np.float16)
    pack_h = np.concatenate([bd, cc0, cc1, bc0, bc1, qy16, qc16],
                            axis=1)  # [128, 656] fp16

    # f32 pack: recip tables [128,8]x2 + bias_y + ls_col
    qy = np.tile((Y_TABLE.T * FACTOR), (16, 1)).astype(np.float32)
    qc = np.tile((C_TABLE.T * FACTOR), (16, 1)).astype(np.float32)
    ry = (1.0 / qy).astype(np.float32)
    rc = (1.0 / qc).astype(np.float32)
    bias_y = np.zeros((128, 1), dtype=np.float32)
    bias_y[0::8, 0] = np.float32(LSC)
    ls_col = np.zeros((128, 1), dtype=np.float32)
    ls_col[0::8, 0] = np.float32(-LS2D)
    pack_f = np.concatenate([ry, rc, bias_y, ls_col], axis=1)  # [128, 18]

    return (np.ascontiguousarray(pack_r, dtype=np.float32),
            np.ascontiguousarray(pack_h, dtype=np.float16),
            np.ascontiguousarray(pack_f, dtype=np.float32))


_PACK_R, _PACK_H, _PACK_F = _constants()
_PROGRAM = None
TRACE = False
LAST_RESULT = None


def _build_program():
    import concourse.bacc as bacc
    import concourse.mybir as mybir
    from concourse.tile import TileContext

    f32 = mybir.dt.float32
    f32r = mybir.dt.float32r
    f16 = mybir.dt.float16
    ACT_COPY = mybir.ActivationFunctionType.Copy
    ACT_IDENT = mybir.ActivationFunctionType.Identity
    ADD = mybir.AluOpType.add
    SUB = mybir.AluOpType.subtract
    MULT = mybir.AluOpType.mult
    MIN = mybir.AluOpType.min
    MAX = mybir.AluOpType.max

    nc = bacc.Bacc("TRN2", target_bir_lowering=False, debug=False,
                   num_devices=N_CORES)

    x_d = nc.dram_tensor("xc", [IMGS_PER_CORE, 3, H, W], f16,
                         kind="ExternalInput").ap()
    out_d = nc.dram_tensor("outc", [IMGS_PER_CORE, 3, H, W], f16,
                           kind="ExternalOutput").ap()
    packr_d = nc.dram_tensor("pack_r", list(_PACK_R.shape), f32,
                             kind="ExternalInput").ap()
    packh_d = nc.dram_tensor("pack_h", list(_PACK_H.shape), f16,
                             kind="ExternalInput").ap()
    packf_d = nc.dram_tensor("pack_f", list(_PACK_F.shape), f32,
                             kind="ExternalInput").ap()
    _DBG = os.environ.get("KV_DBG", "0") == "1"
    if _DBG:
        dbg_t1c = nc.dram_tensor("dbg_t1c", [IMGS_PER_CORE, 2, 128, 2, 2,
                                             256], f16,
                                 kind="ExternalOutput").ap()
        dbg_cqc = nc.dram_tensor("dbg_cqc", [IMGS_PER_CORE, 128, 2, 2, 2,
                                             128], f16,
                                 kind="ExternalOutput").ap()
        dbg_t1y = nc.dram_tensor("dbg_t1y", [IMGS_PER_CORE, 128, 4, 512],
                                 f32, kind="ExternalOutput").ap()
        dbg_cqy = nc.dram_tensor("dbg_cqy", [IMGS_PER_CORE, 128, 2, 4, 256],
                                 f16, kind="ExternalOutput").ap()

    def cp_eng(code):
        return {"a": "act", "d": "dve", "p": "pool"}[code]

    with TileContext(nc) as tc:
        with (
            tc.tile_pool(name="const", bufs=1) as cpool,
            tc.tile_pool(name="data", bufs=2) as dpool,
            tc.tile_pool(name="work", bufs=2) as wpool,
            tc.tile_pool(name="psA", bufs=_PSA, space="PSUM") as psA,
            tc.tile_pool(name="psB", bufs=_PSB, space="PSUM") as psB,
            tc.tile_pool(name="psT", bufs=_PST, space="PSUM") as psT,
            tc.tile_pool(name="psD", bufs=_PSD, space="PSUM") as psD,
        ):
            # ---- PE warmup: dummy matmuls while DMAs are in flight ----
            wu0 = cpool.tile([128, 16], f32, name="wu0")
            nc.gpsimd.memset(wu0[:], 1.0)
            wu = cpool.tile([128, 16], f32r, name="wu")
            nc.gpsimd.tensor_copy(wu[:], wu0[:])

            # ---- constant DMAs on ACT queue ----
            cr_t = cpool.tile([128, 384], f32r, name="cr_t")
            nc.scalar.dma_start(cr_t[:], packr_d.bitcast(f32r))
            ch_t = cpool.tile([128, 656], f16, name="ch_t")
            nc.scalar.dma_start(ch_t[:], packh_d)
            cf_t = cpool.tile([128, 18], f32, name="cf_t")
            nc.scalar.dma_start(cf_t[:], packf_d)

            cs = {
                "b_y": cr_t[:, 0:128],
                "b_c_k0": cr_t[:, 128:256],
                "b_c_k1": cr_t[:, 256:384],
                "bd": ch_t[:, 0:128],
                "cc0": ch_t[:, 128:256],
                "cc1": ch_t[:, 256:384],
                "b_c16_k0": ch_t[:, 384:512],
                "b_c16_k1": ch_t[:, 512:640],
                "qy16": ch_t[:, 640:648],
                "qc16": ch_t[:, 648:656],
                "ry": cf_t[:, 0:8],
                "rc": cf_t[:, 8:16],
                "bias_y": cf_t[:, 16:17],
                "ls_col": cf_t[:, 17:18],
            }

            wp = psA.tile([128, 512], f32, name="wp", tag="psa")
            for _ in range(_WARM):
                nc.tensor.matmul(wp[:16, 0:16], wu[:], wu[:], start=True,
                                 stop=True)

            # ---- on-chip generated fp16 consts ----
            # fused A consts: per color one [128, 2(k), 256] tile
            _GN = [0]

            def gmul(dst, src_ap, w):
                e = "dap"[_GN[0] % 3]
                _GN[0] += 1
                if e == "d":
                    nc.vector.tensor_scalar_mul(dst, src_ap, float(w))
                elif e == "a":
                    nc.scalar.activation(dst, src_ap, ACT_COPY,
                                         scale=float(w))
                else:
                    nc.gpsimd.tensor_scalar_mul(dst, src_ap, float(w))

            for ci, wname in enumerate("rgb"):
                act = cpool.tile([128, 2, 256], f16, name=f"ac_{wname}")
                cs[f"ac_{wname}"] = act
                for k in range(2):
                    gmul(act[:, k, 0:128], cs["b_y"],
                         W_FWD["y"][ci] * 255.0)
                    src = (cs["b_c_k0"][:, 0:64] if k == 0
                           else cs["b_c_k1"][:, 64:128])
                    for cni, cn in enumerate(("cb", "cr")):
                        gmul(act[:, k, 128 + 64 * cni:192 + 64 * cni],
                             src, W_FWD[cn][ci] * 255.0)

            def gen16(key, src_ap, factor, width=128):
                t = cpool.tile([128, width], f16, name=f"g_{key}")
                gmul(t[:], src_ap, float(factor))
                cs[key] = t[:]

            gen16("dd_y", cs["bd"], 1.0 / 255.0)
            for och, terms in W_BWD.items():
                for cch, wv in terms.items():
                    for k in (0, 1):
                        gen16(f"d_{och}_{cch}_k{k}", cs[f"cc{k}"],
                              wv / 255.0)

            def bcN(key, reps, w=8):
                ap = cs[key]
                for _ in range(len(reps)):
                    ap = ap.unsqueeze(1)
                return ap.broadcast_to([128, *reps, w])

            def mm(out_ps, lhsT_ap, rhs_ap, start, stop):
                nc.tensor.matmul(out_ps, lhsT_ap, rhs_ap,
                                 start=start, stop=stop)

            def ew_copy(eng, dst, src):
                if eng == "a":
                    nc.scalar.activation(dst, src, ACT_COPY)
                elif eng == "d":
                    nc.vector.tensor_copy(dst, src)
                else:
                    nc.gpsimd.tensor_copy(dst, src)

            S = [{} for _ in range(IMGS_PER_CORE)]

            # ---------------- stages ----------------
            def st_load(img):
                """f16 input, one DMA per (ch, row-half)."""
                x16 = dpool.tile([128, 3, 4, W], f16, name=f"x16_{img}",
                                 tag="x16")
                for h in range(2):
                    for ch in range(3):
                        nc.sync.dma_start(
                            x16[:, ch, 2 * h:2 * h + 2, :],
                            x_d[img, ch, 256 * h:256 * (h + 1), :]
                            .rearrange("(k p) w -> p k w", p=128))
                S[img]["x16"] = x16

            def st_A(img, r, jp):
                """fused A+T1, row-chunk r, col-pair jp.
                PSUM [128, 2(jj), 256 = Y(128)|cb(64)|cr(64)]."""
                x16 = S[img]["x16"]
                h, k = r // 2, r % 2
                t1 = S[img].get("t1y")
                if t1 is None:
                    t1 = wpool.tile([128, 4, W], f32r, name=f"t1y_{img}",
                                    tag="t1y", bufs=2)
                    S[img]["t1y"] = t1
                t1c = S[img].setdefault("t1c", {}).get(jp)
                if t1c is None:
                    t1c = wpool.tile([128, 2, 2, 256], f16,
                                     name=f"t1c_{img}_{jp}", tag="t1c",
                                     bufs=4)
                    S[img]["t1c"][jp] = t1c
                pa0 = psA.tile([128, 512], f32, name=f"A_{img}_{r}_{jp}",
                               tag="psa")
                pa = pa0[:].rearrange("p (a b) -> p a b", a=2)
                for jj in range(2):
                    j = 2 * jp + jj
                    for ci, wname in enumerate("rgb"):
                        mm(pa[:, jj, :],
                           x16[:, ci, r, 128 * j:128 * (j + 1)],
                           cs[f"ac_{wname}"][:, k, :], ci == 0, ci == 2)
                # Y part [128,2,128] -> t1y[:, 2jp:2jp+2, 128r:+128]
                ew_copy(_AEVY, t1[:, 2 * jp:2 * jp + 2,
                                 128 * r:128 * (r + 1)], pa[:, :, 0:128])
                # chroma [128,2,2,64] -> t1c[:, jj, cn, 128h+64k:+64]
                ew_copy(_AEVC,
                        t1c[:, :, :, 128 * h + 64 * k:128 * h + 64 * k + 64],
                        pa[:, :, 128:256].rearrange("p a (c u) -> p a c u",
                                                    c=2))

            def st_By(img, g, h):
                """col-DCT Y for i-pair g, row-half h -> PSUM [128,2,256]."""
                t1 = S[img]["t1y"]
                pb = S[img].setdefault("pby", {}).get(g)
                if pb is None:
                    pb = psB.tile([128, 2, 256], f32, name=f"By_{img}_{g}",
                                  tag="psb")
                    S[img]["pby"][g] = pb
                for ii in range(2):
                    i = 2 * g + ii
                    mm(pb[:, ii, 256 * h // 256:1, :][:, 0, :]
                       if False else
                       pb[:].rearrange("p a (hh b) -> p a hh b", hh=2)
                       [:, ii, h, :],
                       cs["b_y"], t1[:, i, 256 * h:256 * (h + 1)],
                       True, True)

            def st_Bc(img, h):
                """chroma col-DCT both cn, row-half h -> [128,2,2,128]."""
                t1c = S[img]["t1c"]
                pb = S[img].get("pbc")
                if pb is None:
                    pb = psB.tile([128, 2, 2, 128], f32, name=f"Bc_{img}",
                                  tag="psb")
                    S[img]["pbc"] = pb
                for cni in range(2):
                    for k in range(2):  # jj parity, shared LDW
                        for b in range(2):
                            mm(pb[:].rearrange(
                                "p c (hh b) -> p c hh b", hh=2)
                               [:, cni, h, 64 * b:64 * (b + 1)]
                               if False else
                               pb[:, cni, b, :],
                               cs[f"b_c16_k{k}"],
                               t1c[b][:, k, cni, 128 * h:128 * (h + 1)],
                               k == 0, k == 1)

            def st_Qy(img, g):
                """quant Y i-pair g: [128,2,256] PSUM -> cq f16."""
                pb = S[img]["pby"].pop(g)
                cq = S[img].get("cqy")
                if cq is None:
                    cq = wpool.tile([128, 4, W], f16, name=f"cqy_{img}",
                                    tag="cqy", bufs=2)
                    S[img]["cqy"] = cq
                pv = pb[:].rearrange("p a (c b) -> p a c b", b=8)
                w1 = wpool.tile([128, 512], f16, name=f"q1y_{img}_{g}",
                                tag="q1", bufs=4)
                w1v = w1[:].rearrange("p (a c b) -> p a c b", a=2, b=8)
                # AC columns: plain *recip ; DC columns: (x-1024)*recip
                nc.vector.tensor_tensor(w1v[:, :, :, 1:8], pv[:, :, :, 1:8],
                                        bcN("ry", (2, 32), 7)[:, :, :, 1:8]
                                        if False else
                                        bcN("ry", (2, 32))[:, :, :, 1:8],
                                        MULT)
                nc.vector.scalar_tensor_tensor(
                    w1v[:, :, :, 0], pv[:, :, :, 0], cs["ls_col"],
                    bcN("ry", (2, 32))[:, :, :, 0], ADD, MULT)
                w2 = wpool.tile([128, 512], f16, name=f"q2y_{img}_{g}",
                                tag="q2", bufs=4)
                nc.vector.tensor_scalar(w2[:], w1[:], float(MAGIC),
                                        float(MAGIC), ADD, SUB)
                cqv = cq[:, 2 * g:2 * g + 2, :]\
                    .rearrange("p a (c b) -> p a c b", b=8)
                w2v = w1[:].rearrange("p (a c b) -> p a c b", a=2, b=8)
                w2v = w2[:].rearrange("p (a c b) -> p a c b", a=2, b=8)
                nc.vector.tensor_tensor(cqv, w2v, bcN("qy16", (2, 32)),
                                        MULT)

            def st_Qc(img):
                """quant chroma both cn: [128,2,2,128] PSUM -> cq f16."""
                pb = S[img].pop("pbc")
                cq = S[img].get("cqc")
                if cq is None:
                    cq = wpool.tile([128, 2, 2, 256], f16,
                                    name=f"cqc_{img}", tag="cqc", bufs=2)
                    S[img]["cqc"] = cq
                pv = pb[:].rearrange("p c a (u b) -> p c a u b", b=8)
                w1 = wpool.tile([128, 512], f16, name=f"q1c_{img}",
                                tag="q1", bufs=4)
                w1v = w1[:].rearrange("p (c a u b) -> p c a u b",
                                      c=2, a=2, b=8)
                nc.vector.tensor_tensor(w1v[:, :, :, :, 1:8],
                                        pv[:, :, :, :, 1:8],
                                        bcN("rc", (2, 2, 16))[:, :, :, :,
                                                              1:8], MULT)
                nc.vector.scalar_tensor_tensor(
                    w1v[:, :, :, :, 0], pv[:, :, :, :, 0], cs["ls_col"],
                    bcN("rc", (2, 2, 16))[:, :, :, :, 0], ADD, MULT)
                w2 = wpool.tile([128, 512], f16, name=f"q2c_{img}",
                                tag="q2", bufs=4)
                nc.vector.tensor_scalar(w2[:], w1[:], float(MAGIC),
                                        float(MAGIC), ADD, SUB)
                # cq layout [128, cn, b, (h,k... u' 256)]: write whole
                cqv = cq[:].rearrange("p c a (u b) -> p c a u b", b=8)
                w2v = w2[:].rearrange("p (c a u b) -> p c a u b",
                                      c=2, a=2, b=8)
                nc.vector.tensor_tensor(cqv, w2v, bcN("qc16", (2, 2, 16)),
                                        MULT)

            def st_CT2y(img, j):
                cq = S[img]["cqy"]
                pt = psT.tile([128, W], f32, name=f"CT2y_{img}_{j}",
                              tag="psT")
                for i in range(4):
                    mm(pt[:, 128 * i:128 * (i + 1)],
                       cq[:, i, 128 * j:128 * (j + 1)], cs["bd"],
                       True, True)
                t2 = wpool.tile([128, W], f16, name=f"t2y_{img}_{j}",
                                tag="t2y", bufs=8)
                if _CEVY == "a":
                    nc.scalar.activation(t2[:], pt[:], ACT_IDENT,
                                         bias=cs["bias_y"], scale=1.0)
                else:
                    nc.vector.tensor_tensor(
                        t2[:], pt[:],
                        cs["bias_y"].broadcast_to([128, W]), ADD)
                S[img].setdefault("t2y", {})[j] = t2

            def st_CT2c(img, cn, j):
                cq = S[img]["cqc"]
                cni = ("cb", "cr").index(cn)
                pt = psT.tile([128, W], f32, name=f"CT2c_{img}_{cn}_{j}",
                              tag="psT")
                for i in range(4):
                    mm(pt[:, 128 * i:128 * (i + 1)],
                       cq[:, cni, i // 2, 128 * j:128 * (j + 1)],
                       cs[f"cc{i % 2}"], True, True)
                t2 = wpool.tile([128, W], f16, name=f"t2c_{img}_{cn}_{j}",
                                tag="t2c", bufs=8)
                ew_copy(_CEVC, t2[:], pt[:])
                S[img].setdefault("t2c", {})[(cn, j)] = t2

            def st_D(img, och, chunks):
                """vertical IDCT + color fold; grouped-LDW emission:
                all chunks of one term before the next term."""
                t2y = S[img]["t2y"]
                t2c = S[img]["t2c"]
                oi = "rgb".index(och)
                ot = S[img].get(f"ot_{och}")
                if ot is None:
                    ot = dpool.tile([128, 4, W], f16,
                                    name=f"ot_{img}_{och}", tag="ot",
                                    bufs=3)
                    S[img][f"ot_{och}"] = ot
                terms = list(W_BWD[och].items())
                pds = {}
                for i in chunks:
                    pds[i] = psD.tile([128, W], f32,
                                      name=f"D_{img}_{och}_{i}", tag="psD")
                for i in chunks:  # dd_y shared LDW across chunks
                    mm(pds[i][:], cs["dd_y"], t2y[i][:], True, False)
                for ti, (cch, _) in enumerate(terms):
                    last = ti == len(terms) - 1
                    for i in chunks:  # same k-parity -> shared LDW
                        mm(pds[i][:], cs[f"d_{och}_{cch}_k{i % 2}"],
                           t2c[(cch, i // 2)][:], False, last)
                for i in chunks:
                    fe_n = img * 12 + oi * 4 + i
                    eng = _FINPAT[fe_n % len(_FINPAT)]
                    if eng == "r":
                        # ACT relu (PSUM->SBUF) then Pool min (SBUF only)
                        fe = wpool.tile([128, W], f16,
                                        name=f"fin_{img}_{och}_{i}",
                                        tag="fin", bufs=2)
                        nc.scalar.activation(
                            fe[:], pds[i][:],
                            mybir.ActivationFunctionType.Relu)
                        nc.gpsimd.tensor_scalar_min(ot[:, i, :], fe[:], 1.0)
                    else:
                        nc.vector.tensor_scalar(ot[:, i, :], pds[i][:],
                                                1.0, 0.0, MIN, MAX)
                    nc.sync.dma_start(
                        out_d[img, oi, 128 * i:128 * (i + 1), :],
                        ot[:, i, :])

            # ---------------- emission schedule ----------------
            st_load(0)
            st_load(1)

            def half_front(img, h):
                for r in (2 * h, 2 * h + 1):
                    for jp in range(2):
                        st_A(img, r, jp)
                for g in range(2):
                    st_By(img, g, h)
                st_Bc(img, h)
                st_Qy(img, 0, h)
                st_Qy(img, 1, h)
                st_Qc(img, h)

            def half_back(img, h):
                st_CT2y(img, 2 * h)
                st_CT2y(img, 2 * h + 1)
                st_CT2c(img, "cr", h)
                st_CT2c(img, "cb", h)
                st_D(img, "r", (2 * h, 2 * h + 1))
                st_D(img, "g", (2 * h, 2 * h + 1))
                st_D(img, "b", (2 * h, 2 * h + 1))

            for tok in _HS.split("."):
                fn = half_front if tok[0] == "f" else half_back
                fn(int(tok[1]), int(tok[2]))

            if _DBG:
                for img in range(IMGS_PER_CORE):
                    for jp in range(2):
                        nc.sync.dma_start(dbg_t1c[img, jp],
                                          S[img]["t1c"][jp][:])
                    nc.sync.dma_start(dbg_cqc[img], S[img]["cqc"][:])
                    nc.sync.dma_start(dbg_t1y[img],
                                      S[img]["t1y"][:].bitcast(f32))
                    nc.sync.dma_start(dbg_cqy[img], S[img]["cqy"][:])

    nc.compile()
    return nc


def kernel(x: np.ndarray) -> np.ndarray:
    global _PROGRAM, LAST_RESULT
    from concourse.bass_utils import run_bass_kernel_spmd

    x = np.ascontiguousarray(np.asarray(x, dtype=np.float16))
    assert x.shape == (N_CORES * IMGS_PER_CORE, 3, H, W)

    if _PROGRAM is None:
        _PROGRAM = _build_program()
    nc = _PROGRAM

    in_maps = []
    for c in range(N_CORES):
        m = {"xc": x[IMGS_PER_CORE * c:IMGS_PER_CORE * (c + 1)],
             "pack_r": _PACK_R, "pack_h": _PACK_H, "pack_f": _PACK_F}
        in_maps.append(m)

    res = run_bass_kernel_spmd(nc, in_maps, list(range(N_CORES)), trace=TRACE)
    LAST_RESULT = res
    out = np.concatenate([res.results[c]["outc"] for c in range(N_CORES)],
                         axis=0)
    return np.ascontiguousarray(out, dtype=np.float32)


def debug_run(x):
    """Run with debug dumps enabled; returns core-0 results dict."""
    global _PROGRAM
    os.environ["KV_DBG"] = "1"
    from concourse.bass_utils import run_bass_kernel_spmd
    x = np.ascontiguousarray(np.asarray(x, dtype=np.float16))
    prog = _build_program()
    in_maps = []
    for c in range(N_CORES):
        m = {"xc": x[IMGS_PER_CORE * c:IMGS_PER_CORE * (c + 1)],
             "pack_r": _PACK_R, "pack_h": _PACK_H, "pack_f": _PACK_F}
        in_maps.append(m)
    res = run_bass_kernel_spmd(prog, in_maps, list(range(N_CORES)))
    return res.results[0]


# revision 24
# speedup vs baseline: 1.1813x; 1.0414x over previous
"""DiffJPEG (quality=75) Bass kernel for Trainium2, 8-core data-parallel.

v3 pipeline per image (f16 I/O, fused stages, cost-model-shaped):
  host:  input cast f32->f16, output cast f16->f32 (halves DMA traffic).
  load:  6 DMAs per image [128,2,512] f16 straight into A-stage layout.
  A+T1:  per (row-chunk r, col-pair jp) one PSUM group [128,512] =
         2x[Y(128)|cb(64)|cr(64)]; 6 fp16 MMs (3 colors x 2 j),
         rhs = fused per-color consts [128,256]. Evac: one Y copy
         [128,2,128] -> t1Y, one chroma copy [128,2,2,64] -> t1c.
  B:     col-DCT f32r(Y)/f16(chroma) into [128,512] PSUM groups
         (i-pairs for Y, both-cn for chroma).
  Q:     q1 = P*recip (DVE tt, split DC columns via scalar_tensor_tensor
         with per-partition -1024 level-shift offset), q2 = f32-magic
         round (DVE ts, 4x f16), q3 = *q (DVE tt, 2x f16).
  C+T2:  t2 = cq.T @ IDCT-consts per block (fp16), +LS bias on Y evac.
  D:     col-IDCT + color + upsample folds, consts pre-scaled 1/255;
         fp16 matmuls, 512-wide, grouped-LDW emission.
  fin:   clip via single (min 1, max 0) tensor_scalar per chunk spread
         across DVE/Pool, then f16 DMA out.
"""
import os
import sys

sys.path.insert(0, "/opt/trn_rl_repo")

import numpy as np

_WARM = int(os.environ.get("KV_WARM", "60"))
_FINPAT = os.environ.get("KV_FIN", "d")
_AEVY = os.environ.get("KV_AEVY", "a")     # A-evac Y copy engine
_AEVC = os.environ.get("KV_AEVC", "a")     # A-evac chroma copy engine
_CEVY = os.environ.get("KV_CEVY", "a")     # C-evac Y engine
_CEVC = os.environ.get("KV_CEVC", "a")     # C-evac chroma engine
_PSA = int(os.environ.get("KV_PSA", "3"))
_PSB = int(os.environ.get("KV_PSB", "1"))
_PST = int(os.environ.get("KV_PST", "2"))
_PSD = int(os.environ.get("KV_PSD", "2"))
_HS = os.environ.get("KV_HS", "f00.f01.f10.b00.b01.f11.b10.b11")

QUALITY = 75
FACTOR = (200.0 - 2.0 * QUALITY) / 100.0  # 0.5
MAGIC = np.float32(1.5 * 2.0 ** 23)
LS2D = 1024.0  # 2D-DCT of the -128 level shift, lands on (DC,DC) coefs
LSC = np.float64(128.0 * 8.0 * 0.5 / np.sqrt(2.0))  # +128 recon bias (C)

Y_TABLE = np.array([
    [16, 11, 10, 16, 24, 40, 51, 61],
    [12, 12, 14, 19, 26, 58, 60, 55],
    [14, 13, 16, 24, 40, 57, 69, 56],
    [14, 17, 22, 29, 51, 87, 80, 62],
    [18, 22, 37, 56, 68, 109, 103, 77],
    [24, 35, 55, 64, 81, 104, 113, 92],
    [49, 64, 78, 87, 103, 121, 120, 101],
    [72, 92, 95, 98, 112, 100, 103, 99]], dtype=np.float64)

C_TABLE = np.array([
    [17, 18, 24, 47, 99, 99, 99, 99],
    [18, 21, 26, 66, 99, 99, 99, 99],
    [24, 26, 56, 99, 99, 99, 99, 99],
    [47, 66, 99, 99, 99, 99, 99, 99],
    [99, 99, 99, 99, 99, 99, 99, 99],
    [99, 99, 99, 99, 99, 99, 99, 99],
    [99, 99, 99, 99, 99, 99, 99, 99],
    [99, 99, 99, 99, 99, 99, 99, 99]], dtype=np.float64)

W_FWD = {
    "y": (0.299, 0.587, 0.114),
    "cb": (-0.168736, -0.331264, 0.5),
    "cr": (0.5, -0.418688, -0.081312),
}
W_BWD = {
    "r": {"cr": 1.402},
    "g": {"cb": -0.344136, "cr": -0.714136},
    "b": {"cb": 1.772},
}

N_CORES = 8
IMGS_PER_CORE = 2
H = W = 512


def _round_f32r(x):
    """Round f32 to the 12-explicit-mantissa-bit f32r grid (RNE)."""
    x = np.ascontiguousarray(x, dtype=np.float32)
    u = x.view(np.uint32).astype(np.uint64)
    drop = 11
    half = np.uint64(1 << (drop - 1))
    low = u & np.uint64((1 << drop) - 1)
    u_hi = u >> np.uint64(drop)
    up = (low > half) | ((low == half) & ((u_hi & np.uint64(1)) == 1))
    u2 = (u_hi + up.astype(np.uint64)) << np.uint64(drop)
    return (u2 & np.uint64(0xFFFFFFFF)).astype(np.uint32).view(np.float32)


def _dct_mat():
    xg = np.arange(8, dtype=np.float64)
    ug = np.arange(8, dtype=np.float64)
    Dm = 0.5 * np.cos((2.0 * xg[None, :] + 1.0) * ug[:, None] * np.pi / 16.0)
    Dm[0, :] *= 1.0 / np.sqrt(2.0)
    return Dm


def _constants():
    D8 = _dct_mat()
    BD128 = np.kron(np.eye(16), D8)  # [128,128]
    P = np.zeros((128, 256))
    idx = np.arange(128)
    P[idx, 2 * idx] = 0.5
    P[idx, 2 * idx + 1] = 0.5
    M = np.kron(np.eye(16), D8) @ P  # [128, 256] row-pool + DCT
    P0, P1 = M[:, :128], M[:, 128:]

    # f32r pack: B-stage stationary for Y + A-const sources
    b_y = _round_f32r(BD128.T)
    b_c_k0 = _round_f32r(P0.T)
    b_c_k1 = _round_f32r(P1.T)
    pack_r = np.concatenate([b_y, b_c_k0, b_c_k1], axis=1)  # [128, 384]

    # fp16 pack: CT2 moving consts + B chroma stationaries + q tables
    bd = np.asarray(BD128, dtype=np.float16)
    cc0 = np.asarray(2.0 * P0, dtype=np.float16)
    cc1 = np.asarray(2.0 * P1, dtype=np.float16)
    bc0 = np.asarray(P0.T, dtype=np.float16)
    bc1 = np.asarray(P1.T, dtype=np.float16)
    qy16 = np.tile((Y_TABLE.T * FACTOR), (16, 1)).astype(np.float16)
    qc16 = np.tile((C_TABLE.T * FACTOR), (16, 1)).astype(np.float16)
    pack_h = np.concatenate([bd, cc0, cc1, bc0, bc1, qy16, qc16],
                            axis=1)  # [128, 656] fp16

    # f32 pack: recip tables [128,8]x2 + bias_y + ls_col
    qy = np.tile((Y_TABLE.T * FACTOR), (16, 1)).astype(np.float32)
    qc = np.tile((C_TABLE.T * FACTOR), (16, 1)).astype(np.float32)
    ry = (1.0 / qy).astype(np.float32)
    rc = (1.0 / qc).astype(np.float32)
    bias_y = np.zeros((128, 1), dtype=np.float32)
    bias_y[0::8, 0] = np.float32(LSC)
    ls_col = np.zeros((128, 1), dtype=np.float32)
    ls_col[0::8, 0] = np.float32(-LS2D)
    pack_f = np.concatenate([ry, rc, bias_y, ls_col], axis=1)  # [128, 18]

    return (np.ascontiguousarray(pack_r, dtype=np.float32),
            np.ascontiguousarray(pack_h, dtype=np.float16),
            np.ascontiguousarray(pack_f, dtype=np.float32))


_PACK_R, _PACK_H, _PACK_F = _constants()
_PROGRAM = None
TRACE = False
LAST_RESULT = None


def _build_program():
    import concourse.bacc as bacc
    import concourse.mybir as mybir
    from concourse.tile import TileContext

    f32 = mybir.dt.float32
    f32r = mybir.dt.float32r
    f16 = mybir.dt.float16
    ACT_COPY = mybir.ActivationFunctionType.Copy
    ACT_IDENT = mybir.ActivationFunctionType.Identity
    ADD = mybir.AluOpType.add
    SUB = mybir.AluOpType.subtract
    MULT = mybir.AluOpType.mult
    MIN = mybir.AluOpType.min
    MAX = mybir.AluOpType.max

    nc = bacc.Bacc("TRN2", target_bir_lowering=False, debug=False,
                   num_devices=N_CORES)

    x_d = nc.dram_tensor("xc", [IMGS_PER_CORE, 3, H, W], f16,
                         kind="ExternalInput").ap()
    out_d = nc.dram_tensor("outc", [IMGS_PER_CORE, 3, H, W], f16,
                           kind="ExternalOutput").ap()
    packr_d = nc.dram_tensor("pack_r", list(_PACK_R.shape), f32,
                             kind="ExternalInput").ap()
    packh_d = nc.dram_tensor("pack_h", list(_PACK_H.shape), f16,
                             kind="ExternalInput").ap()
    packf_d = nc.dram_tensor("pack_f", list(_PACK_F.shape), f32,
                             kind="ExternalInput").ap()
    _DBG = os.environ.get("KV_DBG", "0") == "1"
    if _DBG:
        dbg_t1c = nc.dram_tensor("dbg_t1c", [IMGS_PER_CORE, 2, 128, 2, 2,
                                             256], f16,
                                 kind="ExternalOutput").ap()
        dbg_cqc = nc.dram_tensor("dbg_cqc", [IMGS_PER_CORE, 128, 2, 2, 2,
                                             128], f16,
                                 kind="ExternalOutput").ap()
        dbg_t1y = nc.dram_tensor("dbg_t1y", [IMGS_PER_CORE, 128, 4, 512],
                                 f32, kind="ExternalOutput").ap()
        dbg_cqy = nc.dram_tensor("dbg_cqy", [IMGS_PER_CORE, 128, 2, 4, 256],
                                 f16, kind="ExternalOutput").ap()

    def cp_eng(code):
        return {"a": "act", "d": "dve", "p": "pool"}[code]

    with TileContext(nc) as tc:
        with (
            tc.tile_pool(name="const", bufs=1) as cpool,
            tc.tile_pool(name="data", bufs=2) as dpool,
            tc.tile_pool(name="work", bufs=2) as wpool,
            tc.tile_pool(name="psA", bufs=_PSA, space="PSUM") as psA,
            tc.tile_pool(name="psB", bufs=_PSB, space="PSUM") as psB,
            tc.tile_pool(name="psT", bufs=_PST, space="PSUM") as psT,
            tc.tile_pool(name="psD", bufs=_PSD, space="PSUM") as psD,
        ):
            # ---- PE warmup: dummy matmuls while DMAs are in flight ----
            wu0 = cpool.tile([128, 16], f32, name="wu0")
            nc.gpsimd.memset(wu0[:], 1.0)
            wu = cpool.tile([128, 16], f32r, name="wu")
            nc.gpsimd.tensor_copy(wu[:], wu0[:])

            # ---- constant DMAs on ACT queue ----
            cr_t = cpool.tile([128, 384], f32r, name="cr_t")
            nc.scalar.dma_start(cr_t[:], packr_d.bitcast(f32r))
            ch_t = cpool.tile([128, 656], f16, name="ch_t")
            nc.scalar.dma_start(ch_t[:], packh_d)
            cf_t = cpool.tile([128, 18], f32, name="cf_t")
            nc.scalar.dma_start(cf_t[:], packf_d)

            cs = {
                "b_y": cr_t[:, 0:128],
                "b_c_k0": cr_t[:, 128:256],
                "b_c_k1": cr_t[:, 256:384],
                "bd": ch_t[:, 0:128],
                "cc0": ch_t[:, 128:256],
                "cc1": ch_t[:, 256:384],
                "b_c16_k0": ch_t[:, 384:512],
                "b_c16_k1": ch_t[:, 512:640],
                "qy16": ch_t[:, 640:648],
                "qc16": ch_t[:, 648:656],
                "ry": cf_t[:, 0:8],
                "rc": cf_t[:, 8:16],
                "bias_y": cf_t[:, 16:17],
                "ls_col": cf_t[:, 17:18],
            }

            wp = psA.tile([128, 512], f32, name="wp", tag="psa")
            for _ in range(_WARM):
                nc.tensor.matmul(wp[:16, 0:16], wu[:], wu[:], start=True,
                                 stop=True)

            # ---- on-chip generated fp16 consts ----
            # fused A consts: per color one [128, 2(k), 256] tile
            _GN = [0]

            def gmul(dst, src_ap, w):
                e = "dap"[_GN[0] % 3]
                _GN[0] += 1
                if e == "d":
                    nc.vector.tensor_scalar_mul(dst, src_ap, float(w))
                elif e == "a":
                    nc.scalar.activation(dst, src_ap, ACT_COPY,
                                         scale=float(w))
                else:
                    nc.gpsimd.tensor_scalar_mul(dst, src_ap, float(w))

            for ci, wname in enumerate("rgb"):
                act = cpool.tile([128, 2, 256], f16, name=f"ac_{wname}")
                cs[f"ac_{wname}"] = act
                for k in range(2):
                    gmul(act[:, k, 0:128], cs["b_y"],
                         W_FWD["y"][ci] * 255.0)
                    src = (cs["b_c_k0"][:, 0:64] if k == 0
                           else cs["b_c_k1"][:, 64:128])
                    for cni, cn in enumerate(("cb", "cr")):
                        gmul(act[:, k, 128 + 64 * cni:192 + 64 * cni],
                             src, W_FWD[cn][ci] * 255.0)

            def gen16(key, src_ap, factor, width=128):
                t = cpool.tile([128, width], f16, name=f"g_{key}")
                gmul(t[:], src_ap, float(factor))
                cs[key] = t[:]

            gen16("dd_y", cs["bd"], 1.0 / 255.0)
            for och, terms in W_BWD.items():
                for cch, wv in terms.items():
                    for k in (0, 1):
                        gen16(f"d_{och}_{cch}_k{k}", cs[f"cc{k}"],
                              wv / 255.0)

            def bcN(key, reps, w=8):
                ap = cs[key]
                for _ in range(len(reps)):
                    ap = ap.unsqueeze(1)
                return ap.broadcast_to([128, *reps, w])

            def mm(out_ps, lhsT_ap, rhs_ap, start, stop):
                nc.tensor.matmul(out_ps, lhsT_ap, rhs_ap,
                                 start=start, stop=stop)

            def ew_copy(eng, dst, src):
                if eng == "a":
                    nc.scalar.activation(dst, src, ACT_COPY)
                elif eng == "d":
                    nc.vector.tensor_copy(dst, src)
                else:
                    nc.gpsimd.tensor_copy(dst, src)

            S = [{} for _ in range(IMGS_PER_CORE)]

            # ---------------- stages ----------------
            def st_load(img):
                """f16 input, one DMA per (ch, row-half)."""
                x16 = dpool.tile([128, 3, 4, W], f16, name=f"x16_{img}",
                                 tag="x16")
                for h in range(2):
                    for ch in range(3):
                        nc.sync.dma_start(
                            x16[:, ch, 2 * h:2 * h + 2, :],
                            x_d[img, ch, 256 * h:256 * (h + 1), :]
                            .rearrange("(k p) w -> p k w", p=128))
                S[img]["x16"] = x16

            def st_A(img, r, jp):
                """fused A+T1, row-chunk r, col-pair jp.
                PSUM [128, 2(jj), 256 = Y(128)|cb(64)|cr(64)]."""
                x16 = S[img]["x16"]
                h, k = r // 2, r % 2
                t1 = S[img].get("t1y")
                if t1 is None:
                    t1 = wpool.tile([128, 4, W], f32r, name=f"t1y_{img}",
                                    tag="t1y", bufs=2)
                    S[img]["t1y"] = t1
                t1c = S[img].setdefault("t1c", {}).get(jp)
                if t1c is None:
                    t1c = wpool.tile([128, 2, 2, 256], f16,
                                     name=f"t1c_{img}_{jp}", tag="t1c",
                                     bufs=4)
                    S[img]["t1c"][jp] = t1c
                pa0 = psA.tile([128, 512], f32, name=f"A_{img}_{r}_{jp}",
                               tag="psa")
                pa = pa0[:].rearrange("p (a b) -> p a b", a=2)
                for jj in range(2):
                    j = 2 * jp + jj
                    for ci, wname in enumerate("rgb"):
                        mm(pa[:, jj, :],
                           x16[:, ci, r, 128 * j:128 * (j + 1)],
                           cs[f"ac_{wname}"][:, k, :], ci == 0, ci == 2)
                # Y part [128,2,128] -> t1y[:, 2jp:2jp+2, 128r:+128]
                ew_copy(_AEVY, t1[:, 2 * jp:2 * jp + 2,
                                 128 * r:128 * (r + 1)], pa[:, :, 0:128])
                # chroma [128,2,2,64] -> t1c[:, jj, cn, 128h+64k:+64]
                ew_copy(_AEVC,
                        t1c[:, :, :, 128 * h + 64 * k:128 * h + 64 * k + 64],
                        pa[:, :, 128:256].rearrange("p a (c u) -> p a c u",
                                                    c=2))

            def st_By(img, g, h):
                """col-DCT Y for i-pair g, row-half h -> PSUM [128,2,256]."""
                t1 = S[img]["t1y"]
                pb = S[img].setdefault("pby", {}).get(g)
                if pb is None:
                    pb = psB.tile([128, 2, 256], f32, name=f"By_{img}_{g}",
                                  tag="psb")
                    S[img]["pby"][g] = pb
                for ii in range(2):
                    i = 2 * g + ii
                    mm(pb[:, ii, 256 * h // 256:1, :][:, 0, :]
                       if False else
                       pb[:].rearrange("p a (hh b) -> p a hh b", hh=2)
                       [:, ii, h, :],
                       cs["b_y"], t1[:, i, 256 * h:256 * (h + 1)],
                       True, True)

            def st_Bc(img, h):
                """chroma col-DCT both cn, row-half h -> [128,2,2,128]."""
                t1c = S[img]["t1c"]
                pb = S[img].get("pbc")
                if pb is None:
                    pb = psB.tile([128, 2, 2, 128], f32, name=f"Bc_{img}",
                                  tag="psb")
                    S[img]["pbc"] = pb
                for cni in range(2):
                    for k in range(2):  # jj parity, shared LDW
                        for b in range(2):
                            mm(pb[:].rearrange(
                                "p c (hh b) -> p c hh b", hh=2)
                               [:, cni, h, 64 * b:64 * (b + 1)]
                               if False else
                               pb[:, cni, b, :],
                               cs[f"b_c16_k{k}"],
                               t1c[b][:, k, cni, 128 * h:128 * (h + 1)],
                               k == 0, k == 1)

            def st_Qy(img, g):
                """quant Y i-pair g: [128,2,256] PSUM -> cq f16."""
                pb = S[img]["pby"].pop(g)
                cq = S[img].get("cqy")
                if cq is None:
                    cq = wpool.tile([128, 4, W], f16, name=f"cqy_{img}",
                                    tag="cqy", bufs=2)
                    S[img]["cqy"] = cq
                pv = pb[:].rearrange("p a (c b) -> p a c b", b=8)
                w1 = wpool.tile([128, 512], f16, name=f"q1y_{img}_{g}",
                                tag="q1", bufs=4)
                w1v = w1[:].rearrange("p (a c b) -> p a c b", a=2, b=8)
                # AC columns: plain *recip ; DC columns: (x-1024)*recip
                nc.vector.tensor_tensor(w1v[:, :, :, 1:8], pv[:, :, :, 1:8],
                                        bcN("ry", (2, 32), 7)[:, :, :, 1:8]
                                        if False else
                                        bcN("ry", (2, 32))[:, :, :, 1:8],
                                        MULT)
                nc.vector.scalar_tensor_tensor(
                    w1v[:, :, :, 0], pv[:, :, :, 0], cs["ls_col"],
                    bcN("ry", (2, 32))[:, :, :, 0], ADD, MULT)
                w2 = wpool.tile([128, 512], f16, name=f"q2y_{img}_{g}",
                                tag="q2", bufs=4)
                nc.vector.tensor_scalar(w2[:], w1[:], float(MAGIC),
                                        float(MAGIC), ADD, SUB)
                cqv = cq[:, 2 * g:2 * g + 2, :]\
                    .rearrange("p a (c b) -> p a c b", b=8)
                w2v = w1[:].rearrange("p (a c b) -> p a c b", a=2, b=8)
                w2v = w2[:].rearrange("p (a c b) -> p a c b", a=2, b=8)
                nc.vector.tensor_tensor(cqv, w2v, bcN("qy16", (2, 32)),
                                        MULT)

            def st_Qc(img):
                """quant chroma both cn: [128,2,2,128] PSUM -> cq f16."""
                pb = S[img].pop("pbc")
                cq = S[img].get("cqc")
                if cq is None:
                    cq = wpool.tile([128, 2, 2, 256], f16,
                                    name=f"cqc_{img}", tag="cqc", bufs=2)
                    S[img]["cqc"] = cq
                pv = pb[:].rearrange("p c a (u b) -> p c a u b", b=8)
                w1 = wpool.tile([128, 512], f16, name=f"q1c_{img}",
                                tag="q1", bufs=4)
                w1v = w1[:].rearrange("p (c a u b) -> p c a u b",
                                      c=2, a=2, b=8)
                nc.vector.tensor_tensor(w1v[:, :, :, :, 1:8],
                                        pv[:, :, :, :, 1:8],
                                        bcN("rc", (2, 2, 16))[:, :, :, :,
                                                              1:8], MULT)
                nc.vector.scalar_tensor_tensor(
                    w1v[:, :, :, :, 0], pv[:, :, :, :, 0], cs["ls_col"],
                    bcN("rc", (2, 2, 16))[:, :, :, :, 0], ADD, MULT)
                w2 = wpool.tile([128, 512], f16, name=f"q2c_{img}",
                                tag="q2", bufs=4)
                nc.vector.tensor_scalar(w2[:], w1[:], float(MAGIC),
                                        float(MAGIC), ADD, SUB)
                # cq layout [128, cn, b, (h,k... u' 256)]: write whole
                cqv = cq[:].rearrange("p c a (u b) -> p c a u b", b=8)
                w2v = w2[:].rearrange("p (c a u b) -> p c a u b",
                                      c=2, a=2, b=8)
                nc.vector.tensor_tensor(cqv, w2v, bcN("qc16", (2, 2, 16)),
                                        MULT)

            def st_CT2y(img, j):
                cq = S[img]["cqy"]
                pt = psT.tile([128, W], f32, name=f"CT2y_{img}_{j}",
                              tag="psT")
                for i in range(4):
                    mm(pt[:, 128 * i:128 * (i + 1)],
                       cq[:, i, 128 * j:128 * (j + 1)], cs["bd"],
                       True, True)
                t2 = wpool.tile([128, W], f16, name=f"t2y_{img}_{j}",
                                tag="t2y", bufs=8)
                if _CEVY == "a":
                    nc.scalar.activation(t2[:], pt[:], ACT_IDENT,
                                         bias=cs["bias_y"], scale=1.0)
                else:
                    nc.vector.tensor_tensor(
                        t2[:], pt[:],
                        cs["bias_y"].broadcast_to([128, W]), ADD)
                S[img].setdefault("t2y", {})[j] = t2

            def st_CT2c(img, cn, j):
                cq = S[img]["cqc"]
                cni = ("cb", "cr").index(cn)
                pt = psT.tile([128, W], f32, name=f"CT2c_{img}_{cn}_{j}",
                              tag="psT")
                for i in range(4):
                    mm(pt[:, 128 * i:128 * (i + 1)],
                       cq[:, cni, i // 2, 128 * j:128 * (j + 1)],
                       cs[f"cc{i % 2}"], True, True)
                t2 = wpool.tile([128, W], f16, name=f"t2c_{img}_{cn}_{j}",
                                tag="t2c", bufs=8)
                ew_copy(_CEVC, t2[:], pt[:])
                S[img].setdefault("t2c", {})[(cn, j)] = t2

            def st_D(img, och, chunks):
                """vertical IDCT + color fold; grouped-LDW emission:
                all chunks of one term before the next term."""
                t2y = S[img]["t2y"]
                t2c = S[img]["t2c"]
                oi = "rgb".index(och)
                ot = S[img].get(f"ot_{och}")
                if ot is None:
                    ot = dpool.tile([128, 4, W], f16,
                                    name=f"ot_{img}_{och}", tag="ot",
                                    bufs=3)
                    S[img][f"ot_{och}"] = ot
                terms = list(W_BWD[och].items())
                pds = {}
                for i in chunks:
                    pds[i] = psD.tile([128, W], f32,
                                      name=f"D_{img}_{och}_{i}", tag="psD")
                for i in chunks:  # dd_y shared LDW across chunks
                    mm(pds[i][:], cs["dd_y"], t2y[i][:], True, False)
                for ti, (cch, _) in enumerate(terms):
                    last = ti == len(terms) - 1
                    for i in chunks:  # same k-parity -> shared LDW
                        mm(pds[i][:], cs[f"d_{och}_{cch}_k{i % 2}"],
                           t2c[(cch, i // 2)][:], False, last)
                for i in chunks:
                    fe_n = img * 12 + oi * 4 + i
                    eng = _FINPAT[fe_n % len(_FINPAT)]
                    if eng == "r":
                        # ACT relu (PSUM->SBUF) then Pool min (SBUF only)
                        fe = wpool.tile([128, W], f16,
                                        name=f"fin_{img}_{och}_{i}",
                                        tag="fin", bufs=2)
                        nc.scalar.activation(
                            fe[:], pds[i][:],
                            mybir.ActivationFunctionType.Relu)
                        nc.gpsimd.tensor_scalar_min(ot[:, i, :], fe[:], 1.0)
                    else:
                        nc.vector.tensor_scalar(ot[:, i, :], pds[i][:],
                                                1.0, 0.0, MIN, MAX)
                    nc.sync.dma_start(
                        out_d[img, oi, 128 * i:128 * (i + 1), :],
                        ot[:, i, :])

            # ---------------- emission schedule ----------------
            st_load(0)
            st_load(1)

            def half_front(img, h):
                for r in (2 * h, 2 * h + 1):
                    for jp in range(2):
                        st_A(img, r, jp)
                for g in range(2):
                    st_By(img, g, h)
                st_Bc(img, h)
                st_Qy(img, 0, h)
                st_Qy(img, 1, h)
                st_Qc(img, h)

            def half_back(img, h):
                st_CT2y(img, 2 * h)
                st_CT2y(img, 2 * h + 1)
                st_CT2c(img, "cr", h)
                st_CT2c(img, "cb", h)
                st_D(img, "r", (2 * h, 2 * h + 1))
                st_D(img, "g", (2 * h, 2 * h + 1))
                st_D(img, "b", (2 * h, 2 * h + 1))

            for tok in _HS.split("."):
                fn = half_front if tok[0] == "f" else half_back
                fn(int(tok[1]), int(tok[2]))

            if _DBG:
                for img in range(IMGS_PER_CORE):
                    for jp in range(2):
                        nc.sync.dma_start(dbg_t1c[img, jp],
                                          S[img]["t1c"][jp][:])
                    nc.sync.dma_start(dbg_cqc[img], S[img]["cqc"][:])
                    nc.sync.dma_start(dbg_t1y[img],
                                      S[img]["t1y"][:].bitcast(f32))
                    nc.sync.dma_start(dbg_cqy[img], S[img]["cqy"][:])

    nc.compile()
    return nc


def kernel(x: np.ndarray) -> np.ndarray:
    global _PROGRAM, LAST_RESULT
    from concourse.bass_utils import run_bass_kernel_spmd

    x = np.ascontiguousarray(np.asarray(x, dtype=np.float16))
    assert x.shape == (N_CORES * IMGS_PER_CORE, 3, H, W)

    if _PROGRAM is None:
        _PROGRAM = _build_program()
    nc = _PROGRAM

    in_maps = []
    for c in range(N_CORES):
        m = {"xc": x[IMGS_PER_CORE * c:IMGS_PER_CORE * (c + 1)],
             "pack_r": _PACK_R, "pack_h": _PACK_H, "pack_f": _PACK_F}
        in_maps.append(m)

    res = run_bass_kernel_spmd(nc, in_maps, list(range(N_CORES)), trace=TRACE)
    LAST_RESULT = res
    out = np.concatenate([res.results[c]["outc"] for c in range(N_CORES)],
                         axis=0)
    return np.ascontiguousarray(out, dtype=np.float32)


def debug_run(x):
    """Run with debug dumps enabled; returns core-0 results dict."""
    global _PROGRAM
    os.environ["KV_DBG"] = "1"
    from concourse.bass_utils import run_bass_kernel_spmd
    x = np.ascontiguousarray(np.asarray(x, dtype=np.float16))
    prog = _build_program()
    in_maps = []
    for c in range(N_CORES):
        m = {"xc": x[IMGS_PER_CORE * c:IMGS_PER_CORE * (c + 1)],
             "pack_r": _PACK_R, "pack_h": _PACK_H, "pack_f": _PACK_F}
        in_maps.append(m)
    res = run_bass_kernel_spmd(prog, in_maps, list(range(N_CORES)))
    return res.results[0]


# revision 28
# speedup vs baseline: 1.1862x; 1.0041x over previous
"""DiffJPEG (quality=75) Bass kernel for Trainium2, 8-core data-parallel.

v3 pipeline per image (f16 I/O, fused stages, cost-model-shaped):
  host:  input cast f32->f16, output cast f16->f32 (halves DMA traffic).
  load:  6 DMAs per image [128,2,512] f16 straight into A-stage layout.
  A+T1:  per (row-chunk r, col-pair jp) one PSUM group [128,512] =
         2x[Y(128)|cb(64)|cr(64)]; 6 fp16 MMs (3 colors x 2 j),
         rhs = fused per-color consts [128,256]. Evac: one Y copy
         [128,2,128] -> t1Y, one chroma copy [128,2,2,64] -> t1c.
  B:     col-DCT f32r(Y)/f16(chroma) into [128,512] PSUM groups
         (i-pairs for Y, both-cn for chroma).
  Q:     q1 = P*recip (DVE tt, split DC columns via scalar_tensor_tensor
         with per-partition -1024 level-shift offset), q2 = f32-magic
         round (DVE ts, 4x f16), q3 = *q (DVE tt, 2x f16).
  C+T2:  t2 = cq.T @ IDCT-consts per block (fp16), +LS bias on Y evac.
  D:     col-IDCT + color + upsample folds, consts pre-scaled 1/255;
         fp16 matmuls, 512-wide, grouped-LDW emission.
  fin:   clip via single (min 1, max 0) tensor_scalar per chunk spread
         across DVE/Pool, then f16 DMA out.
"""
import os
import sys

sys.path.insert(0, "/opt/trn_rl_repo")

import numpy as np

_WARM = int(os.environ.get("KV_WARM", "60"))
_FINPAT = os.environ.get("KV_FIN", "d")
_AEVY = os.environ.get("KV_AEVY", "a")     # A-evac Y copy engine
_AEVC = os.environ.get("KV_AEVC", "a")     # A-evac chroma copy engine
_CEVY = os.environ.get("KV_CEVY", "a")     # C-evac Y engine
_CEVC = os.environ.get("KV_CEVC", "a")     # C-evac chroma engine
_PSA = int(os.environ.get("KV_PSA", "3"))
_PSB = int(os.environ.get("KV_PSB", "1"))
_PST = int(os.environ.get("KV_PST", "2"))
_PSD = int(os.environ.get("KV_PSD", "2"))
_HS = os.environ.get("KV_HS", "f00.f01.b00.f10.b01.f11.b10.b11")

QUALITY = 75
FACTOR = (200.0 - 2.0 * QUALITY) / 100.0  # 0.5
MAGIC = np.float32(1.5 * 2.0 ** 23)
LS2D = 1024.0  # 2D-DCT of the -128 level shift, lands on (DC,DC) coefs
LSC = np.float64(128.0 * 8.0 * 0.5 / np.sqrt(2.0))  # +128 recon bias (C)

Y_TABLE = np.array([
    [16, 11, 10, 16, 24, 40, 51, 61],
    [12, 12, 14, 19, 26, 58, 60, 55],
    [14, 13, 16, 24, 40, 57, 69, 56],
    [14, 17, 22, 29, 51, 87, 80, 62],
    [18, 22, 37, 56, 68, 109, 103, 77],
    [24, 35, 55, 64, 81, 104, 113, 92],
    [49, 64, 78, 87, 103, 121, 120, 101],
    [72, 92, 95, 98, 112, 100, 103, 99]], dtype=np.float64)

C_TABLE = np.array([
    [17, 18, 24, 47, 99, 99, 99, 99],
    [18, 21, 26, 66, 99, 99, 99, 99],
    [24, 26, 56, 99, 99, 99, 99, 99],
    [47, 66, 99, 99, 99, 99, 99, 99],
    [99, 99, 99, 99, 99, 99, 99, 99],
    [99, 99, 99, 99, 99, 99, 99, 99],
    [99, 99, 99, 99, 99, 99, 99, 99],
    [99, 99, 99, 99, 99, 99, 99, 99]], dtype=np.float64)

W_FWD = {
    "y": (0.299, 0.587, 0.114),
    "cb": (-0.168736, -0.331264, 0.5),
    "cr": (0.5, -0.418688, -0.081312),
}
W_BWD = {
    "r": {"cr": 1.402},
    "g": {"cb": -0.344136, "cr": -0.714136},
    "b": {"cb": 1.772},
}

N_CORES = 8
IMGS_PER_CORE = 2
H = W = 512


def _round_f32r(x):
    """Round f32 to the 12-explicit-mantissa-bit f32r grid (RNE)."""
    x = np.ascontiguousarray(x, dtype=np.float32)
    u = x.view(np.uint32).astype(np.uint64)
    drop = 11
    half = np.uint64(1 << (drop - 1))
    low = u & np.uint64((1 << drop) - 1)
    u_hi = u >> np.uint64(drop)
    up = (low > half) | ((low == half) & ((u_hi & np.uint64(1)) == 1))
    u2 = (u_hi + up.astype(np.uint64)) << np.uint64(drop)
    return (u2 & np.uint64(0xFFFFFFFF)).astype(np.uint32).view(np.float32)


def _dct_mat():
    xg = np.arange(8, dtype=np.float64)
    ug = np.arange(8, dtype=np.float64)
    Dm = 0.5 * np.cos((2.0 * xg[None, :] + 1.0) * ug[:, None] * np.pi / 16.0)
    Dm[0, :] *= 1.0 / np.sqrt(2.0)
    return Dm


def _constants():
    D8 = _dct_mat()
    BD128 = np.kron(np.eye(16), D8)  # [128,128]
    P = np.zeros((128, 256))
    idx = np.arange(128)
    P[idx, 2 * idx] = 0.5
    P[idx, 2 * idx + 1] = 0.5
    M = np.kron(np.eye(16), D8) @ P  # [128, 256] row-pool + DCT
    P0, P1 = M[:, :128], M[:, 128:]

    # f32r pack: B-stage stationary for Y + A-const sources
    b_y = _round_f32r(BD128.T)
    b_c_k0 = _round_f32r(P0.T)
    b_c_k1 = _round_f32r(P1.T)
    pack_r = np.concatenate([b_y, b_c_k0, b_c_k1], axis=1)  # [128, 384]

    # fp16 pack: CT2 moving consts + B chroma stationaries + q tables
    bd = np.asarray(BD128, dtype=np.float16)
    cc0 = np.asarray(2.0 * P0, dtype=np.float16)
    cc1 = np.asarray(2.0 * P1, dtype=np.float16)
    bc0 = np.asarray(P0.T, dtype=np.float16)
    bc1 = np.asarray(P1.T, dtype=np.float16)
    qy16 = np.tile((Y_TABLE.T * FACTOR), (16, 1)).astype(np.float16)
    qc16 = np.tile((C_TABLE.T * FACTOR), (16, 1)).astype(np.float16)
    pack_h = np.concatenate([bd, cc0, cc1, bc0, bc1, qy16, qc16],
                            axis=1)  # [128, 656] fp16

    # f32 pack: recip tables [128,8]x2 + bias_y + ls_col
    qy = np.tile((Y_TABLE.T * FACTOR), (16, 1)).astype(np.float32)
    qc = np.tile((C_TABLE.T * FACTOR), (16, 1)).astype(np.float32)
    ry = (1.0 / qy).astype(np.float32)
    rc = (1.0 / qc).astype(np.float32)
    bias_y = np.zeros((128, 1), dtype=np.float32)
    bias_y[0::8, 0] = np.float32(LSC)
    ls_col = np.zeros((128, 1), dtype=np.float32)
    ls_col[0::8, 0] = np.float32(-LS2D)
    pack_f = np.concatenate([ry, rc, bias_y, ls_col], axis=1)  # [128, 18]

    return (np.ascontiguousarray(pack_r, dtype=np.float32),
            np.ascontiguousarray(pack_h, dtype=np.float16),
            np.ascontiguousarray(pack_f, dtype=np.float32))


_PACK_R, _PACK_H, _PACK_F = _constants()
_PROGRAM = None
TRACE = False
LAST_RESULT = None


def _build_program():
    import concourse.bacc as bacc
    import concourse.mybir as mybir
    from concourse.tile import TileContext

    f32 = mybir.dt.float32
    f32r = mybir.dt.float32r
    f16 = mybir.dt.float16
    ACT_COPY = mybir.ActivationFunctionType.Copy
    ACT_IDENT = mybir.ActivationFunctionType.Identity
    ADD = mybir.AluOpType.add
    SUB = mybir.AluOpType.subtract
    MULT = mybir.AluOpType.mult
    MIN = mybir.AluOpType.min
    MAX = mybir.AluOpType.max

    nc = bacc.Bacc("TRN2", target_bir_lowering=False, debug=False,
                   num_devices=N_CORES)

    x_d = nc.dram_tensor("xc", [IMGS_PER_CORE, 3, H, W], f16,
                         kind="ExternalInput").ap()
    out_d = nc.dram_tensor("outc", [IMGS_PER_CORE, 3, H, W], f16,
                           kind="ExternalOutput").ap()
    packr_d = nc.dram_tensor("pack_r", list(_PACK_R.shape), f32,
                             kind="ExternalInput").ap()
    packh_d = nc.dram_tensor("pack_h", list(_PACK_H.shape), f16,
                             kind="ExternalInput").ap()
    packf_d = nc.dram_tensor("pack_f", list(_PACK_F.shape), f32,
                             kind="ExternalInput").ap()
    _DBG = os.environ.get("KV_DBG", "0") == "1"
    if _DBG:
        dbg_t1c = nc.dram_tensor("dbg_t1c", [IMGS_PER_CORE, 2, 128, 2, 2,
                                             256], f16,
                                 kind="ExternalOutput").ap()
        dbg_cqc = nc.dram_tensor("dbg_cqc", [IMGS_PER_CORE, 128, 2, 2, 2,
                                             128], f16,
                                 kind="ExternalOutput").ap()
        dbg_t1y = nc.dram_tensor("dbg_t1y", [IMGS_PER_CORE, 128, 4, 512],
                                 f32, kind="ExternalOutput").ap()
        dbg_cqy = nc.dram_tensor("dbg_cqy", [IMGS_PER_CORE, 128, 2, 4, 256],
                                 f16, kind="ExternalOutput").ap()

    def cp_eng(code):
        return {"a": "act", "d": "dve", "p": "pool"}[code]

    with TileContext(nc) as tc:
        with (
            tc.tile_pool(name="const", bufs=1) as cpool,
            tc.tile_pool(name="data", bufs=2) as dpool,
            tc.tile_pool(name="work", bufs=2) as wpool,
            tc.tile_pool(name="psA", bufs=_PSA, space="PSUM") as psA,
            tc.tile_pool(name="psB", bufs=_PSB, space="PSUM") as psB,
            tc.tile_pool(name="psT", bufs=_PST, space="PSUM") as psT,
            tc.tile_pool(name="psD", bufs=_PSD, space="PSUM") as psD,
        ):
            # ---- PE warmup: dummy matmuls while DMAs are in flight ----
            wu0 = cpool.tile([128, 16], f32, name="wu0")
            nc.gpsimd.memset(wu0[:], 1.0)
            wu = cpool.tile([128, 16], f32r, name="wu")
            nc.gpsimd.tensor_copy(wu[:], wu0[:])

            # ---- constant DMAs on ACT queue ----
            _CQD = os.environ.get("KV_CQD", "a")
            _cdma = {"g": nc.gpsimd.dma_start, "a": nc.scalar.dma_start,
                     "s": nc.sync.dma_start}[_CQD]
            cr_t = cpool.tile([128, 384], f32r, name="cr_t")
            _cdma(cr_t[:], packr_d.bitcast(f32r))
            ch_t = cpool.tile([128, 656], f16, name="ch_t")
            _cdma(ch_t[:], packh_d)
            cf_t = cpool.tile([128, 18], f32, name="cf_t")
            _cdma(cf_t[:], packf_d)

            cs = {
                "b_y": cr_t[:, 0:128],
                "b_c_k0": cr_t[:, 128:256],
                "b_c_k1": cr_t[:, 256:384],
                "bd": ch_t[:, 0:128],
                "cc0": ch_t[:, 128:256],
                "cc1": ch_t[:, 256:384],
                "b_c16_k0": ch_t[:, 384:512],
                "b_c16_k1": ch_t[:, 512:640],
                "qy16": ch_t[:, 640:648],
                "qc16": ch_t[:, 648:656],
                "ry": cf_t[:, 0:8],
                "rc": cf_t[:, 8:16],
                "bias_y": cf_t[:, 16:17],
                "ls_col": cf_t[:, 17:18],
            }

            wp = psA.tile([128, 512], f32, name="wp", tag="psa")
            for _ in range(_WARM):
                nc.tensor.matmul(wp[:16, 0:16], wu[:], wu[:], start=True,
                                 stop=True)

            # ---- on-chip generated fp16 consts ----
            # fused A consts: per color one [128, 2(k), 256] tile
            _GN = [0]

            def gmul(dst, src_ap, w):
                e = "dap"[_GN[0] % 3]
                _GN[0] += 1
                if e == "d":
                    nc.vector.tensor_scalar_mul(dst, src_ap, float(w))
                elif e == "a":
                    nc.scalar.activation(dst, src_ap, ACT_COPY,
                                         scale=float(w))
                else:
                    nc.gpsimd.tensor_scalar_mul(dst, src_ap, float(w))

            for ci, wname in enumerate("rgb"):
                act = cpool.tile([128, 2, 256], f16, name=f"ac_{wname}")
                cs[f"ac_{wname}"] = act
                for k in range(2):
                    gmul(act[:, k, 0:128], cs["b_y"],
                         W_FWD["y"][ci] * 255.0)
                    src = (cs["b_c_k0"][:, 0:64] if k == 0
                           else cs["b_c_k1"][:, 64:128])
                    for cni, cn in enumerate(("cb", "cr")):
                        gmul(act[:, k, 128 + 64 * cni:192 + 64 * cni],
                             src, W_FWD[cn][ci] * 255.0)

            def gen16(key, src_ap, factor, width=128):
                t = cpool.tile([128, width], f16, name=f"g_{key}")
                gmul(t[:], src_ap, float(factor))
                cs[key] = t[:]

            gen16("dd_y", cs["bd"], 1.0 / 255.0)
            for och, terms in W_BWD.items():
                for cch, wv in terms.items():
                    for k in (0, 1):
                        gen16(f"d_{och}_{cch}_k{k}", cs[f"cc{k}"],
                              wv / 255.0)

            def bcN(key, reps, w=8):
                ap = cs[key]
                for _ in range(len(reps)):
                    ap = ap.unsqueeze(1)
                return ap.broadcast_to([128, *reps, w])

            def mm(out_ps, lhsT_ap, rhs_ap, start, stop):
                nc.tensor.matmul(out_ps, lhsT_ap, rhs_ap,
                                 start=start, stop=stop)

            def ew_copy(eng, dst, src):
                if eng == "a":
                    nc.scalar.activation(dst, src, ACT_COPY)
                elif eng == "d":
                    nc.vector.tensor_copy(dst, src)
                else:
                    nc.gpsimd.tensor_copy(dst, src)

            S = [{} for _ in range(IMGS_PER_CORE)]

            # ---------------- stages ----------------
            def st_load(img):
                """f16 input, one DMA per (ch, row-half)."""
                x16 = dpool.tile([128, 3, 4, W], f16, name=f"x16_{img}",
                                 tag="x16")
                for h in range(2):
                    for ch in range(3):
                        nc.sync.dma_start(
                            x16[:, ch, 2 * h:2 * h + 2, :],
                            x_d[img, ch, 256 * h:256 * (h + 1), :]
                            .rearrange("(k p) w -> p k w", p=128))
                S[img]["x16"] = x16

            def st_A(img, r, jp):
                """fused A+T1, row-chunk r, col-pair jp.
                PSUM [128, 2(jj), 256 = Y(128)|cb(64)|cr(64)]."""
                x16 = S[img]["x16"]
                h, k = r // 2, r % 2
                t1 = S[img].get("t1y")
                if t1 is None:
                    t1 = wpool.tile([128, 4, W], f32r, name=f"t1y_{img}",
                                    tag="t1y", bufs=2)
                    S[img]["t1y"] = t1
                t1c = S[img].setdefault("t1c", {}).get(jp)
                if t1c is None:
                    t1c = wpool.tile([128, 2, 2, 256], f16,
                                     name=f"t1c_{img}_{jp}", tag="t1c",
                                     bufs=4)
                    S[img]["t1c"][jp] = t1c
                pa0 = psA.tile([128, 512], f32, name=f"A_{img}_{r}_{jp}",
                               tag="psa")
                pa = pa0[:].rearrange("p (a b) -> p a b", a=2)
                for jj in range(2):
                    j = 2 * jp + jj
                    for ci, wname in enumerate("rgb"):
                        mm(pa[:, jj, :],
                           x16[:, ci, r, 128 * j:128 * (j + 1)],
                           cs[f"ac_{wname}"][:, k, :], ci == 0, ci == 2)
                # Y part [128,2,128] -> t1y[:, 2jp:2jp+2, 128r:+128]
                ew_copy(_AEVY, t1[:, 2 * jp:2 * jp + 2,
                                 128 * r:128 * (r + 1)], pa[:, :, 0:128])
                # chroma [128,2,2,64] -> t1c[:, jj, cn, 128h+64k:+64]
                ew_copy(_AEVC,
                        t1c[:, :, :, 128 * h + 64 * k:128 * h + 64 * k + 64],
                        pa[:, :, 128:256].rearrange("p a (c u) -> p a c u",
                                                    c=2))

            def st_By(img, g, h):
                """col-DCT Y for i-pair g, row-half h -> PSUM [128,2,256]."""
                t1 = S[img]["t1y"]
                pb = S[img].setdefault("pby", {}).get(g)
                if pb is None:
                    pb = psB.tile([128, 2, 256], f32, name=f"By_{img}_{g}",
                                  tag="psb")
                    S[img]["pby"][g] = pb
                for ii in range(2):
                    i = 2 * g + ii
                    mm(pb[:, ii, 256 * h // 256:1, :][:, 0, :]
                       if False else
                       pb[:].rearrange("p a (hh b) -> p a hh b", hh=2)
                       [:, ii, h, :],
                       cs["b_y"], t1[:, i, 256 * h:256 * (h + 1)],
                       True, True)

            def st_Bc(img, h):
                """chroma col-DCT both cn, row-half h -> [128,2,2,128]."""
                t1c = S[img]["t1c"]
                pb = S[img].get("pbc")
                if pb is None:
                    pb = psB.tile([128, 2, 2, 128], f32, name=f"Bc_{img}",
                                  tag="psb")
                    S[img]["pbc"] = pb
                for cni in range(2):
                    for k in range(2):  # jj parity, shared LDW
                        for b in range(2):
                            mm(pb[:].rearrange(
                                "p c (hh b) -> p c hh b", hh=2)
                               [:, cni, h, 64 * b:64 * (b + 1)]
                               if False else
                               pb[:, cni, b, :],
                               cs[f"b_c16_k{k}"],
                               t1c[b][:, k, cni, 128 * h:128 * (h + 1)],
                               k == 0, k == 1)

            def st_Qy(img, g):
                """quant Y i-pair g: [128,2,256] PSUM -> cq f16."""
                pb = S[img]["pby"].pop(g)
                cq = S[img].get("cqy")
                if cq is None:
                    cq = wpool.tile([128, 4, W], f16, name=f"cqy_{img}",
                                    tag="cqy", bufs=2)
                    S[img]["cqy"] = cq
                pv = pb[:].rearrange("p a (c b) -> p a c b", b=8)
                w1 = wpool.tile([128, 512], f16, name=f"q1y_{img}_{g}",
                                tag="q1", bufs=4)
                w1v = w1[:].rearrange("p (a c b) -> p a c b", a=2, b=8)
                # AC columns: plain *recip ; DC columns: (x-1024)*recip
                nc.vector.tensor_tensor(w1v[:, :, :, 1:8], pv[:, :, :, 1:8],
                                        bcN("ry", (2, 32), 7)[:, :, :, 1:8]
                                        if False else
                                        bcN("ry", (2, 32))[:, :, :, 1:8],
                                        MULT)
                nc.vector.scalar_tensor_tensor(
                    w1v[:, :, :, 0], pv[:, :, :, 0], cs["ls_col"],
                    bcN("ry", (2, 32))[:, :, :, 0], ADD, MULT)
                w2 = wpool.tile([128, 512], f16, name=f"q2y_{img}_{g}",
                                tag="q2", bufs=4)
                nc.vector.tensor_scalar(w2[:], w1[:], float(MAGIC),
                                        float(MAGIC), ADD, SUB)
                cqv = cq[:, 2 * g:2 * g + 2, :]\
                    .rearrange("p a (c b) -> p a c b", b=8)
                w2v = w1[:].rearrange("p (a c b) -> p a c b", a=2, b=8)
                w2v = w2[:].rearrange("p (a c b) -> p a c b", a=2, b=8)
                nc.vector.tensor_tensor(cqv, w2v, bcN("qy16", (2, 32)),
                                        MULT)

            def st_Qc(img):
                """quant chroma both cn: [128,2,2,128] PSUM -> cq f16."""
                pb = S[img].pop("pbc")
                cq = S[img].get("cqc")
                if cq is None:
                    cq = wpool.tile([128, 2, 2, 256], f16,
                                    name=f"cqc_{img}", tag="cqc", bufs=2)
                    S[img]["cqc"] = cq
                pv = pb[:].rearrange("p c a (u b) -> p c a u b", b=8)
                w1 = wpool.tile([128, 512], f16, name=f"q1c_{img}",
                                tag="q1", bufs=4)
                w1v = w1[:].rearrange("p (c a u b) -> p c a u b",
                                      c=2, a=2, b=8)
                nc.vector.tensor_tensor(w1v[:, :, :, :, 1:8],
                                        pv[:, :, :, :, 1:8],
                                        bcN("rc", (2, 2, 16))[:, :, :, :,
                                                              1:8], MULT)
                nc.vector.scalar_tensor_tensor(
                    w1v[:, :, :, :, 0], pv[:, :, :, :, 0], cs["ls_col"],
                    bcN("rc", (2, 2, 16))[:, :, :, :, 0], ADD, MULT)
                w2 = wpool.tile([128, 512], f16, name=f"q2c_{img}",
                                tag="q2", bufs=4)
                nc.vector.tensor_scalar(w2[:], w1[:], float(MAGIC),
                                        float(MAGIC), ADD, SUB)
                # cq layout [128, cn, b, (h,k... u' 256)]: write whole
                cqv = cq[:].rearrange("p c a (u b) -> p c a u b", b=8)
                w2v = w2[:].rearrange("p (c a u b) -> p c a u b",
                                      c=2, a=2, b=8)
                nc.vector.tensor_tensor(cqv, w2v, bcN("qc16", (2, 2, 16)),
                                        MULT)

            def st_CT2y(img, j):
                cq = S[img]["cqy"]
                pt = psT.tile([128, W], f32, name=f"CT2y_{img}_{j}",
                              tag="psT")
                for i in range(4):
                    mm(pt[:, 128 * i:128 * (i + 1)],
                       cq[:, i, 128 * j:128 * (j + 1)], cs["bd"],
                       True, True)
                t2 = wpool.tile([128, W], f16, name=f"t2y_{img}_{j}",
                                tag="t2y", bufs=8)
                if _CEVY == "a":
                    nc.scalar.activation(t2[:], pt[:], ACT_IDENT,
                                         bias=cs["bias_y"], scale=1.0)
                else:
                    nc.vector.tensor_tensor(
                        t2[:], pt[:],
                        cs["bias_y"].broadcast_to([128, W]), ADD)
                S[img].setdefault("t2y", {})[j] = t2

            def st_CT2c(img, cn, j):
                cq = S[img]["cqc"]
                cni = ("cb", "cr").index(cn)
                pt = psT.tile([128, W], f32, name=f"CT2c_{img}_{cn}_{j}",
                              tag="psT")
                for i in range(4):
                    mm(pt[:, 128 * i:128 * (i + 1)],
                       cq[:, cni, i // 2, 128 * j:128 * (j + 1)],
                       cs[f"cc{i % 2}"], True, True)
                t2 = wpool.tile([128, W], f16, name=f"t2c_{img}_{cn}_{j}",
                                tag="t2c", bufs=8)
                ew_copy(_CEVC, t2[:], pt[:])
                S[img].setdefault("t2c", {})[(cn, j)] = t2

            def st_D(img, och, chunks):
                """vertical IDCT + color fold; grouped-LDW emission:
                all chunks of one term before the next term."""
                t2y = S[img]["t2y"]
                t2c = S[img]["t2c"]
                oi = "rgb".index(och)
                ot = S[img].get(f"ot_{och}")
                if ot is None:
                    ot = dpool.tile([128, 4, W], f16,
                                    name=f"ot_{img}_{och}", tag="ot",
                                    bufs=3)
                    S[img][f"ot_{och}"] = ot
                terms = list(W_BWD[och].items())
                pds = {}
                for i in chunks:
                    pds[i] = psD.tile([128, W], f32,
                                      name=f"D_{img}_{och}_{i}", tag="psD")
                for i in chunks:  # dd_y shared LDW across chunks
                    mm(pds[i][:], cs["dd_y"], t2y[i][:], True, False)
                for ti, (cch, _) in enumerate(terms):
                    last = ti == len(terms) - 1
                    for i in chunks:  # same k-parity -> shared LDW
                        mm(pds[i][:], cs[f"d_{och}_{cch}_k{i % 2}"],
                           t2c[(cch, i // 2)][:], False, last)
                for i in chunks:
                    fe_n = img * 12 + oi * 4 + i
                    eng = _FINPAT[fe_n % len(_FINPAT)]
                    if eng == "r":
                        # ACT relu (PSUM->SBUF) then Pool min (SBUF only)
                        fe = wpool.tile([128, W], f16,
                                        name=f"fin_{img}_{och}_{i}",
                                        tag="fin", bufs=2)
                        nc.scalar.activation(
                            fe[:], pds[i][:],
                            mybir.ActivationFunctionType.Relu)
                        nc.gpsimd.tensor_scalar_min(ot[:, i, :], fe[:], 1.0)
                    else:
                        nc.vector.tensor_scalar(ot[:, i, :], pds[i][:],
                                                1.0, 0.0, MIN, MAX)
                    nc.sync.dma_start(
                        out_d[img, oi, 128 * i:128 * (i + 1), :],
                        ot[:, i, :])

            # ---------------- emission schedule ----------------
            st_load(0)
            st_load(1)

            def half_front(img, h):
                for r in (2 * h, 2 * h + 1):
                    for jp in range(2):
                        st_A(img, r, jp)
                for g in range(2):
                    st_By(img, g, h)
                st_Bc(img, h)
                st_Qy(img, 0, h)
                st_Qy(img, 1, h)
                st_Qc(img, h)

            def half_back(img, h):
                st_CT2y(img, 2 * h)
                st_CT2y(img, 2 * h + 1)
                st_CT2c(img, "cr", h)
                st_CT2c(img, "cb", h)
                st_D(img, "r", (2 * h, 2 * h + 1))
                st_D(img, "g", (2 * h, 2 * h + 1))
                st_D(img, "b", (2 * h, 2 * h + 1))

            for tok in _HS.split("."):
                fn = half_front if tok[0] == "f" else half_back
                fn(int(tok[1]), int(tok[2]))

            if _DBG:
                for img in range(IMGS_PER_CORE):
                    for jp in range(2):
                        nc.sync.dma_start(dbg_t1c[img, jp],
                                          S[img]["t1c"][jp][:])
                    nc.sync.dma_start(dbg_cqc[img], S[img]["cqc"][:])
                    nc.sync.dma_start(dbg_t1y[img],
                                      S[img]["t1y"][:].bitcast(f32))
                    nc.sync.dma_start(dbg_cqy[img], S[img]["cqy"][:])

    nc.compile()
    return nc


def kernel(x: np.ndarray) -> np.ndarray:
    global _PROGRAM, LAST_RESULT
    from concourse.bass_utils import run_bass_kernel_spmd

    x = np.ascontiguousarray(np.asarray(x, dtype=np.float16))
    assert x.shape == (N_CORES * IMGS_PER_CORE, 3, H, W)

    if _PROGRAM is None:
        _PROGRAM = _build_program()
    nc = _PROGRAM

    in_maps = []
    for c in range(N_CORES):
        m = {"xc": x[IMGS_PER_CORE * c:IMGS_PER_CORE * (c + 1)],
             "pack_r": _PACK_R, "pack_h": _PACK_H, "pack_f": _PACK_F}
        in_maps.append(m)

    res = run_bass_kernel_spmd(nc, in_maps, list(range(N_CORES)), trace=TRACE)
    LAST_RESULT = res
    out = np.concatenate([res.results[c]["outc"] for c in range(N_CORES)],
                         axis=0)
    return np.ascontiguousarray(out, dtype=np.float32)


def debug_run(x):
    """Run with debug dumps enabled; returns core-0 results dict."""
    global _PROGRAM
    os.environ["KV_DBG"] = "1"
    from concourse.bass_utils import run_bass_kernel_spmd
    x = np.ascontiguousarray(np.asarray(x, dtype=np.float16))
    prog = _build_program()
    in_maps = []
    for c in range(N_CORES):
        m = {"xc": x[IMGS_PER_CORE * c:IMGS_PER_CORE * (c + 1)],
             "pack_r": _PACK_R, "pack_h": _PACK_H, "pack_f": _PACK_F}
        in_maps.append(m)
    res = run_bass_kernel_spmd(prog, in_maps, list(range(N_CORES)))
    return res.results[0]
